# revision 2
# baseline (speedup 1.0000x reference)
"""Self-contained Trainium2 Bass kernel for the GQA attention module (v2).

Sharding: 4-way head tensor-parallel x 2-way batch data-parallel.
Core c = 4*b + t owns batch b, q-heads [8t..8t+8), kv-heads {2t, 2t+1};
the host sums the 4 TP partials per batch (the "all-reduce after wo").

v2 design (vs the ~455-539us v1 8-way-TP baseline):
  - Projections and attention are interleaved per 512-token block: project
    block nt, then run the two attention units (one per kv head) for
    q-chunk nt.  The PE-dense projection matmuls fill the ACT-heavy
    attention stretches, so the PE never idles long enough for the HAM
    clock gate to re-throttle it to half rate.
  - Causal column restriction: per (kt, qc) score tile only the q columns
    with any unmasked row are computed (scores, exp, PV all restricted).
  - The additive mask is applied on the PE: an identity-weighted matmul
    accumulates the raw mask block into the score PSUM before exp, so exp
    writes straight to the P^T ring and the DVE mask multiplies are gone.
  - The PV accumulator (pso) is copied to SBUF right after the last PV
    matmul, releasing its 4 PSUM banks early so the next unit's PV is
    never blocked (pso pool has bufs=1; all other PSUM transients share
    one 2-deep [128,1024] ring).
  - wo runs per q-chunk (needs both kv-head units' at); its matmuls and
    the normalize chain are deferred closures drained at a paced rate
    during the following projection/attention slots, as in v1.
"""

import sys
import types

sys.path.insert(0, "/opt/trn_rl_repo")

import numpy as np
import ml_dtypes


def _install_axon_hook_shim():
    import antenv

    if "antenv.axon_hooks" in sys.modules:
        return
    m = types.ModuleType("antenv.axon_hooks")
    m._hook = None

    def set_axon_ntff_profile_hook(h):
        m._hook = h

    def get_axon_ntff_profile_hook():
        return m._hook

    m.set_axon_ntff_profile_hook = set_axon_ntff_profile_hook
    m.get_axon_ntff_profile_hook = get_axon_ntff_profile_hook
    sys.modules["antenv.axon_hooks"] = m
    antenv.axon_hooks = m
    try:
        from trn_agent_boot.trn_boot import _ntff_profile_via_ctypes

        hook = _ntff_profile_via_ctypes("/opt/axon/libaxon_pjrt.so")
        if hook is not None:
            m.set_axon_ntff_profile_hook(hook)
    except Exception:
        pass


_install_axon_hook_shim()

import concourse.bass as bass
import concourse.mybir as mybir
import concourse.tile as tile
from concourse.bass_utils import run_bass_kernel_spmd

BF16 = mybir.dt.bfloat16
F32 = mybir.dt.float32

B, S, DIM = 2, 2048, 2048
N_HEADS, N_KV_HEADS, HEAD_DIM = 32, 8, 64
N_CORES = 8
TP = 4  # head-parallel groups
HPC = N_HEADS // TP  # 8 q heads per core
KVPC = N_KV_HEADS // TP  # 2 kv heads per core
NPAIR = HPC // 2  # 4 head pairs per core
TOK = S  # tokens per core (its batch)
NKT = S // 128  # 16 k tiles
NQC = S // 512  # 4 q chunks
NEG_THRESH = -1e4


def _patched_drain_and_barrier(self, tick_clock, wait_clock):
    # walrus (CoreV3) only accepts one sync-wait on the tile exit drain;
    # split the accumulated waits across single-wait nops.
    nc = self.nc
    drain_inst = nc.sync.drain()
    wait_clock.add_sem_waits(
        drain_inst.ins, tile.ScopedClock({None: tick_clock.global_clock})
    )
    si = drain_inst.ins.sync_info
    sw = list(si.on_wait) if si and si.on_wait else []
    if len(sw) > 1:
        si.on_wait = [sw[0]]
        for w in sw[1:]:
            n2 = nc.sync.nop(nofuse=True)
            if n2.ins.sync_info is None:
                n2.ins.sync_info = mybir.SyncInfo(on_wait=[w], on_update=[])
            else:
                n2.ins.sync_info.on_wait = [w]
    nc.all_engine_barrier()
    assert self.sems is not None
    popped = nc._tile_sem_poison_stack.pop()
    assert popped is self._sem_poison
    nc.clear_and_free_semaphores(list(self.sems.allocated().values()))
    nc.all_engine_barrier()


tile.TileContext._drain_and_barrier = _patched_drain_and_barrier


def _split_multi_waits(nc):
    """walrus (this build) accepts at most one sync-wait per instruction;
    move extra waits onto same-engine nops inserted just before."""
    n_split = 0
    for f in nc.m.functions:
        for blk in f.blocks:
            new_insts = []
            for inst in blk.instructions:
                si = getattr(inst, "sync_info", None)
                if si is not None and si.on_wait and len(si.on_wait) > 1:
                    extra = list(si.on_wait[:-1])
                    si.on_wait = [si.on_wait[-1]]
                    for w in extra:
                        nop = mybir.InstNoOp(
                            name=nc.get_next_instruction_name(), ins=[], outs=[]
                        )
                        nop.engine = inst.engine
                        nop.sync_info = mybir.SyncInfo(on_wait=[w], on_update=[])
                        new_insts.append(nop)
                        n_split += 1
                new_insts.append(inst)
            blk.instructions[:] = new_insts
    return n_split


def build_nc(tinfo, mask_total_w):
    """tinfo[kt][qc] = ('n'|'t', c0, c1, moff): fully-masked / take, with
    computed q-col range [c0, 512), mixed mask cols [c0, c1) at maskmix
    offset moff (c1 == c0 means no mask needed)."""
    nc = bass.Bass("TRN2", target_bir_lowering=False, debug=False, num_devices=N_CORES)

    MW = max(mask_total_w, 128)
    # x is host-tiled to [p, nt, kt, tok'] so each 512-token block is one
    # contiguous-per-partition 16 KiB-line DMA
    xt_d = nc.dram_tensor("xt_c", [128, NQC * NKT * 512], BF16, kind="ExternalInput")
    # wq/wkv are host-tiled to [p, kt, m] so per-kt chunks are contiguous
    wq_d = nc.dram_tensor(
        "wq_c", [128, NKT * HPC * HEAD_DIM], BF16, kind="ExternalInput"
    )
    wkv_d = nc.dram_tensor(
        "wkv_c", [128, NKT * KVPC * 2 * HEAD_DIM], BF16, kind="ExternalInput"
    )
    wo_d = nc.dram_tensor("wo_c", [HPC * HEAD_DIM, DIM], BF16, kind="ExternalInput")
    maskmix_d = nc.dram_tensor("maskmix", [128, MW], BF16, kind="ExternalInput")
    cosd_d = nc.dram_tensor("cosd", [128, TOK], BF16, kind="ExternalInput")
    sind_d = nc.dram_tensor("sind", [128, TOK], BF16, kind="ExternalInput")
    perm_d = nc.dram_tensor("perm", [128, 128], BF16, kind="ExternalInput")
    eye128_d = nc.dram_tensor("eye128", [128, 128], BF16, kind="ExternalInput")
    out_d = nc.dram_tensor("out_c", [TOK, DIM], BF16, kind="ExternalOutput")

    with tile.TileContext(nc) as tc:
        with (
            tc.tile_pool(name="persist", bufs=1) as persist,
            tc.tile_pool(name="stream", bufs=2) as stream,
            tc.tile_pool(name="small", bufs=2) as small,
            tc.tile_pool(name="otp", bufs=2) as otp,
            tc.tile_pool(name="bcp", bufs=2) as bcp,
        ):
            # ---- persistent tensors ----
            wq_sb = persist.tile([128, NKT, HPC * HEAD_DIM], BF16, tag="wq")
            wkv_sb = persist.tile([128, NKT, KVPC * 2 * HEAD_DIM], BF16, tag="wkv")
            wo_sb = persist.tile([128, NPAIR, DIM], BF16, tag="wo")
            perm_sb = persist.tile([128, 128], BF16, tag="perm")
            eye_sb = persist.tile([128, 128], BF16, tag="eye")
            cos_sb = persist.tile([128, TOK], BF16, tag="cos")
            sin_sb = persist.tile([128, TOK], BF16, tag="sin")
            mask_sb = persist.tile([128, MW], BF16, tag="mask")
            q_sb = persist.tile([128, NPAIR, TOK], BF16, tag="q")  # Q^T
            kT_sb = persist.tile([128, KVPC, TOK], BF16, tag="kT")  # K^T dup halves
            v_sb = persist.tile([128, KVPC * NKT, 68], BF16, tag="v")  # [V|1|pad]
            at_sb = persist.tile([128, NPAIR, TOK], BF16, tag="at")  # normalized A^T
            # raw A^T ring (row 64 carries the softmax denominators)
            aout_sb = persist.tile([65, 2, 4, 512], BF16, tag="aout")
            rec32_sb = persist.tile([65, 2048], F32, tag="rec32")  # ln d (row 64)
            recb_sb = persist.tile([65, 2048], BF16, tag="recb")  # 1/d (row 64)
            ones_sb = persist.tile([128, 64], BF16, tag="ones")
            NRING = 4
            pring = persist.tile([128, NRING, 2048], BF16, tag="pring")

            # DMA emission in need-order: the first Q matmul needs xblk0[kt]
            # and the mt=0 slice of wq[kt], so those chunks go first.
            xblk0 = stream.tile([128, NKT, 512], BF16, tag="xblk")
            for g in range(4):
                nc.sync.dma_start(
                    xblk0[:, 4 * g : 4 * g + 4, :].rearrange("p t n -> p (t n)"),
                    xt_d[:, g * 2048 : (g + 1) * 2048],
                )
                nc.sync.dma_start(
                    wq_sb[:, 4 * g : 4 * g + 4, :].rearrange("p t n -> p (t n)"),
                    wq_d[:, g * 2048 : (g + 1) * 2048],
                )
                nc.sync.dma_start(
                    wkv_sb[:, 4 * g : 4 * g + 4, :].rearrange("p t n -> p (t n)"),
                    wkv_d[:, g * 1024 : (g + 1) * 1024],
                )
            nc.sync.dma_start(perm_sb[:], perm_d[:])
            nc.sync.dma_start(eye_sb[:], eye128_d[:])
            nc.sync.dma_start(cos_sb[:], cosd_d[:])
            nc.sync.dma_start(sin_sb[:], sind_d[:])
            nc.sync.dma_start(mask_sb[:], maskmix_d[:])
            nc.sync.dma_start(wo_sb[:], wo_d.rearrange("(t p) m -> p t m", p=128))
            nc.gpsimd.memset(v_sb[:, :, 64:65], 1.0)
            nc.gpsimd.memset(ones_sb[:], 1.0)

            # PSUM: one 2-deep [128,1024] ring for every transient (score
            # pairs, projection accumulators, RoPE swaps, V-transpose, wo
            # chunks, 1/d broadcast) + a single 4-bank PV accumulator.
            ps_s_cm = tc.tile_pool(name="ps_s", bufs=2, space="PSUM")
            ps_s = ps_s_cm.__enter__()
            ps_o_cm = tc.tile_pool(name="ps_o", bufs=1, space="PSUM")
            ps_o = ps_o_cm.__enter__()

            # Two deferred-closure queues: light normalize work (pss-tag /
            # DVE) drains anywhere; wo chunks allocate the 4-bank pso slot
            # and may ONLY drain where pso is free (projection tile windows
            # and unit tails) — draining one mid-unit would deadlock the PE
            # queue behind the unit's own pso.
            pend_norm = []
            pend_wo = []

            def drain(k):
                for _ in range(min(k, len(pend_norm))):
                    pend_norm.pop(0)()

            def drain_wo(k):
                for _ in range(min(k, len(pend_wo))):
                    pend_wo.pop(0)()

            # ---------------- projection of one 512-token block ----------
            def project_block(nt, xblk=None):
                cs = slice(nt * 512, (nt + 1) * 512)
                if xblk is None:
                    xblk = stream.tile([128, NKT, 512], BF16, tag="xblk")
                    base = nt * NKT * 512
                    for g in range(4):
                        nc.sync.dma_start(
                            xblk[:, 4 * g : 4 * g + 4, :].rearrange(
                                "p t n -> p (t n)"
                            ),
                            xt_d[:, base + g * 2048 : base + (g + 1) * 2048],
                        )
                cosb = cos_sb[:, cs]
                sinb = sin_sb[:, cs]

                # Projection tiles are software-pipelined: tile t's RoPE tail
                # (perm matmul + muls) is emitted after tile t+1's
                # accumulation matmuls, so the PE never queues behind the
                # PSUM->SBUF copy.  The pair-swap matmul writes into the
                # accumulator tile's unused second bank (no extra slot).
                def rope_q(psq, mt):
                    q_tmp = small.tile([128, 512], BF16, tag="q_tmp")
                    nc.scalar.copy(q_tmp[:], psq[:, 0:512])
                    nc.tensor.matmul(psq[:, 512:1024], perm_sb[:], q_tmp[:])
                    v1 = small.tile([128, 512], BF16, tag="v1")
                    nc.vector.tensor_mul(v1[:], q_tmp[:], cosb)
                    v2 = small.tile([128, 512], BF16, tag="v2")
                    nc.vector.tensor_mul(v2[:], psq[:, 512:1024], sinb)
                    nc.vector.tensor_add(q_sb[:, mt, cs], v1[:], v2[:])

                def rope_kv(pskv, j):
                    kv_tmp = small.tile([128, 512], BF16, tag="kv_tmp")
                    nc.scalar.copy(kv_tmp[:], pskv[:, 0:512])
                    # K RoPE on rows 0:64
                    nc.tensor.matmul(
                        pskv[0:64, 512:1024], perm_sb[0:64, 0:64], kv_tmp[0:64, :]
                    )
                    kv1 = small.tile([64, 512], BF16, tag="kv1")
                    nc.vector.tensor_mul(kv1[:], kv_tmp[0:64, :], cosb[0:64, :])
                    kv2 = small.tile([64, 512], BF16, tag="kv2")
                    nc.vector.tensor_mul(kv2[:], pskv[0:64, 512:1024], sinb[0:64, :])
                    nc.vector.tensor_add(kT_sb[0:64, j, cs], kv1[:], kv2[:])
                    # duplicate K^T into partitions 64..127 (so the row-packed
                    # score matmul pairs get distinct PE row groups)
                    nc.gpsimd.dma_start(kT_sb[64:128, j, cs], kT_sb[0:64, j, cs])

                    # V transpose: rows 64:128 of kv_tmp -> natural V [k, 64]
                    pst = ps_s.tile([128, 4, 64], BF16, tag="pss", name="pst")
                    for jq in range(4):
                        nc.tensor.transpose(
                            pst[:, jq, :],
                            kv_tmp[64:128, jq * 128 : (jq + 1) * 128],
                            eye_sb[64:128, 64:128],
                            tile_position=(64, 0),
                        )
                    rc0 = j * NKT + nt * 4
                    nc.scalar.copy(v_sb[:, rc0 : rc0 + 4, 0:64], pst[:])

                st = {"tail": None}

                def do_tiles(tiles):
                    for kind, idx in tiles:
                        ps = ps_s.tile([128, 1024], F32, tag="pss", name="psp")
                        wsb = wq_sb if kind == "q" else wkv_sb
                        for kt in range(NKT):
                            nc.tensor.matmul(
                                ps[:, 0:512],
                                wsb[:, kt, idx * 128 : (idx + 1) * 128],
                                xblk[:, kt, :],
                                start=(kt == 0),
                                stop=(kt == NKT - 1),
                            )
                        if st["tail"]:
                            st["tail"]()
                        if kind == "q":
                            st["tail"] = lambda ps=ps, idx=idx: rope_q(ps, idx)
                        else:
                            st["tail"] = lambda ps=ps, idx=idx: rope_kv(ps, idx)
                        drain(1)
                        drain_wo(1)

                def flush():
                    if st["tail"]:
                        st["tail"]()
                        st["tail"] = None

                return do_tiles, flush

            # ---------------- one attention unit: (qc, kv head j) --------
            def attn_unit(qc, j, uidx, wo_slots=0):
                acts = [kt for kt in range(NKT) if tinfo[kt][qc][0] != "n"]
                assert acts, "fully-masked q chunk unsupported"
                ring = aout_sb[:, uidx % 2]  # [64, 4, 512]
                u = {"qc": qc, "j": j, "ring": ring, "pso": None}
                # wo chunks on the pso slot may drain here only when this
                # unit follows a projection block (the previous unit's pso
                # has long been released, so the PE queue won't block)
                drain_wo(wo_slots)

                def emit_pv(i):
                    kt = acts[i]
                    c0 = tinfo[kt][qc][1]
                    rc = j * NKT + kt
                    for h in range(4):
                        nc.tensor.matmul(
                            u["pso"][:, h, c0:512],
                            v_sb[:, rc, 0:65],
                            pring[:, i % NRING, h * 512 + c0 : (h + 1) * 512],
                            start=(i == 0),
                            stop=(i == len(acts) - 1),
                        )

                for i, kt in enumerate(acts):
                    cls, c0, c1, moff = tinfo[kt][qc]
                    if i == 0:
                        assert c0 == 0, "first active kt must be full-width"
                    ks = slice(kt * 128, (kt + 1) * 128)
                    mw = c1 - c0
                    # scores for 4 heads: 2 row-group-packed matmul pairs
                    for pair in range(2):
                        mt = 2 * j + pair
                        qs = slice(qc * 512 + c0, (qc + 1) * 512)
                        pss = ps_s.tile([128, 1024], F32, tag="pss", name="pss")
                        nc.tensor.matmul(
                            pss[:, c0:512],
                            kT_sb[0:64, j, ks],
                            q_sb[0:64, mt, qs],
                            tile_position=(0, 0),
                            start=True,
                            stop=(mw == 0),
                        )
                        nc.tensor.matmul(
                            pss[:, 512 + c0 : 1024],
                            kT_sb[64:128, j, ks],
                            q_sb[64:128, mt, qs],
                            tile_position=(64, 0),
                            start=True,
                            stop=(mw == 0),
                        )
                        if mw:
                            # accumulate the raw additive mask block into the
                            # mixed columns on the PE (identity stationary)
                            nc.tensor.matmul(
                                pss[:, c0:c1],
                                eye_sb[:],
                                mask_sb[:, moff : moff + mw],
                                start=False,
                                stop=True,
                            )
                            nc.tensor.matmul(
                                pss[:, 512 + c0 : 512 + c1],
                                eye_sb[:],
                                mask_sb[:, moff : moff + mw],
                                start=False,
                                stop=True,
                            )
                        nc.scalar.activation(
                            pring[:, i % NRING, pair * 1024 : (pair + 1) * 1024]
                            .rearrange("p (t n) -> p t n", t=2)[:, :, c0:512],
                            pss[:].rearrange("p (t n) -> p t n", t=2)[:, :, c0:512],
                            mybir.ActivationFunctionType.Exp,
                        )
                    # drain light closures, paced across the sweep
                    k = (
                        -(-len(pend_norm) // max(1, len(acts) - i))
                        if pend_norm
                        else 0
                    )
                    drain(min(k, 2))
                    if i == 1:
                        u["pso"] = ps_o.tile([65, 4, 512], F32, tag="pso", name="pso")
                    if i >= 2:
                        emit_pv(i - 2)
                if len(acts) == 1:
                    u["pso"] = ps_o.tile([65, 4, 512], F32, tag="pso", name="pso")
                while pend_norm:
                    pend_norm.pop(0)()
                for i in range(max(0, len(acts) - 2), len(acts)):
                    emit_pv(i)

                # release pso quickly: copy raw A^T plus the denominator row
                # (65 partitions) to SBUF, split across DVE and ACT.  The
                # ln/exp reciprocal chain runs later as deferred closures so
                # it never delays the next unit's exps in the ACT FIFO.
                pso = u["pso"]
                nc.vector.tensor_copy(ring[:, :, :], pso[:, :, :])
                return u

            def norm_ops(u):
                """Deferred normalize closures for unit u (reads aout ring)."""
                qc, j, ring = u["qc"], u["j"], u["ring"]
                qcs = slice(qc * 512, (qc + 1) * 512)
                ops = []

                def op_ln():
                    nc.scalar.activation(
                        rec32_sb[64:65, :],
                        ring[64:65, :, :].rearrange("p a n -> p (a n)"),
                        mybir.ActivationFunctionType.Ln,
                    )

                def op_recip():
                    nc.scalar.activation(
                        recb_sb[64:65, :],
                        rec32_sb[64:65, :],
                        mybir.ActivationFunctionType.Exp,
                        scale=-1.0,
                    )

                ops.append(op_ln)
                ops.append(op_recip)
                psbs = [None, None]
                for p in range(2):  # head pair within this kv head
                    ch = 2 * j + p

                    def op_psb(p=p):
                        # partition-broadcast 1/d to 64 rows via a K=1 matmul
                        psb = ps_s.tile([128, 1024], F32, tag="pss", name="psb")
                        psbs[p] = psb
                        for hh in range(2):
                            h = 2 * p + hh
                            nc.tensor.matmul(
                                psb[0:64, hh * 512 : (hh + 1) * 512],
                                ones_sb[64:65, :],
                                recb_sb[64:65, h * 512 : (h + 1) * 512],
                            )

                    def op_mul(p=p, ch=ch):
                        bc = bcp.tile([64, 1024], BF16, tag="bc")
                        nc.vector.tensor_copy(bc[:], psbs[p][0:64, :])
                        nc.vector.tensor_mul(
                            at_sb[0:64, ch, qcs], ring[0:64, 2 * p, :], bc[:, 0:512]
                        )
                        att = small.tile([64, 512], BF16, tag="att")
                        nc.vector.tensor_mul(
                            att[:], ring[0:64, 2 * p + 1, :], bc[:, 512:1024]
                        )
                        nc.gpsimd.dma_start(at_sb[64:128, ch, qcs], att[:])

                    ops.append(op_psb)
                    ops.append(op_mul)
                return ops

            def wo_ops(qc, pool, tag, act_cast=True):
                """wo chunk closures for q-chunk qc (needs both units' at)."""
                base = qc * 512
                ops = []
                for jj in range(4):
                    rs = slice(base + jj * 128, base + (jj + 1) * 128)
                    for half in range(2):

                        def op(rs=rs, half=half):
                            psd = pool.tile([128, 1024], F32, tag=tag, name="psd")
                            for sub in range(2):
                                ntc = half * 2 + sub
                                cs2 = slice(ntc * 512, (ntc + 1) * 512)
                                for ch in range(NPAIR):
                                    nc.tensor.matmul(
                                        psd[:, sub * 512 : (sub + 1) * 512],
                                        at_sb[:, ch, rs],
                                        wo_sb[:, ch, cs2],
                                        start=(ch == 0),
                                        stop=(ch == NPAIR - 1),
                                    )
                            ot = otp.tile([128, 1024], BF16, tag="ot")
                            # alternate the PSUM drain between DVE and ACT so
                            # two casts can be in flight (DVE-only when the
                            # chunks drain mid-attention, where ACT is busy)
                            if half == 1 and act_cast:
                                nc.scalar.copy(ot[:], psd[:])
                            else:
                                nc.vector.tensor_copy(ot[:], psd[:])
                            nc.sync.dma_start(
                                out_d[rs, half * 1024 : (half + 1) * 1024], ot[:]
                            )

                        ops.append(op)
                return ops

            # ---------------- main interleaved schedule ------------------
            # Projections run one block ahead of attention, split in halves
            # around the first attention unit of each chunk so every unit
            # start follows PE-dense projection work (keeps HAM warm and
            # lets the previous unit's exp backlog drain).  Early q-chunks'
            # wo uses the pso slot (drained in projection windows and at
            # post-projection unit starts); late chunks fall back to the
            # shared pss ring and drain mid-unit.
            HALF1 = [("q", 0), ("q", 1), ("q", 2)]
            HALF2 = [("q", 3), ("kv", 0), ("kv", 1)]
            uidx = 0
            do0, fl0 = project_block(0, xblk=xblk0)
            do0(HALF1 + HALF2)
            fl0()
            for qc in range(NQC):
                do = fl = None
                if qc + 1 < NQC:
                    do, fl = project_block(qc + 1)
                    do(HALF1)
                u = attn_unit(qc, 0, uidx, wo_slots=2)
                uidx += 1
                pend_norm.extend(norm_ops(u))
                if do:
                    do(HALF2)
                    fl()
                u = attn_unit(qc, 1, uidx, wo_slots=(2 if do else 0))
                uidx += 1
                pend_norm.extend(norm_ops(u))
                if qc < 2:
                    pend_wo.extend(wo_ops(qc, ps_o, "pso"))
                else:
                    pend_norm.extend(wo_ops(qc, ps_s, "pss", act_cast=False))
            while pend_norm:
                pend_norm.pop(0)()
            while pend_wo:
                pend_wo.pop(0)()

            ps_o_cm.__exit__(None, None, None)
            ps_s_cm.__exit__(None, None, None)
    _split_multi_waits(nc)
    return nc


_NC_CACHE = {}


def _analyze_mask(mask):
    """Per (kt, qc) tile info from the [S, S] additive mask ([q, k]).

    Returns (tinfo, total_w, blocks): tinfo[kt][qc] = (cls, c0, c1, moff);
    blocks is the list of [128, w] transposed mask blocks to concatenate."""
    masked = mask <= NEG_THRESH  # [q, k] bool
    tinfo = [[None] * NQC for _ in range(NKT)]
    blocks = []
    moff = 0

    def add_block(q0, q1, kt):
        blk = np.asarray(
            mask[q0:q1, kt * 128 : (kt + 1) * 128].T, dtype=np.float32
        )
        blk = np.maximum(blk, -30000.0)
        blocks.append(blk)
        return blk.shape[1]

    for kt in range(NKT):
        for qc in range(NQC):
            sub = masked[qc * 512 : (qc + 1) * 512, kt * 128 : (kt + 1) * 128]
            col_all = sub.all(axis=1)  # q col fully masked
            col_any = sub.any(axis=1)
            if col_all.all():
                tinfo[kt][qc] = ("n", 0, 0, 0)
                continue
            c0 = int(np.argmin(col_all))  # first not-fully-masked col
            assert not col_all[c0:].any(), "non-contiguous masked col range"
            if col_any[c0:].any():
                c1 = 512 - int(np.argmax(col_any[::-1]))  # last any-masked col + 1
            else:
                c1 = c0
            if c1 > c0:
                w = add_block(qc * 512 + c0, qc * 512 + c1, kt)
                tinfo[kt][qc] = ("t", c0, c1, moff)
                moff += w
            else:
                tinfo[kt][qc] = ("t", c0, c0, 0)
    # the first active tile of each q chunk must be full width (it carries
    # the PSUM has_written init for scores and PV)
    for qc in range(NQC):
        acts = [kt for kt in range(NKT) if tinfo[kt][qc][0] != "n"]
        assert acts, "fully-masked q chunk unsupported"
        kt0 = acts[0]
        cls, c0, c1, _ = tinfo[kt0][qc]
        if c0 != 0:
            c1 = max(c1, c0)
            w = add_block(qc * 512, qc * 512 + c1, kt0)
            tinfo[kt0][qc] = ("t", 0, c1, moff)
            moff += w
    return tinfo, moff, blocks


def _prep_inputs(x, freqs_cos, freqs_sin, mask, wq, wk, wv, wo, blocks, total_w):
    bf = ml_dtypes.bfloat16
    x32 = np.asarray(x, dtype=np.float32)  # [B, S, DIM]

    cos = np.asarray(freqs_cos, dtype=np.float32)  # [S, 32]
    sin = np.asarray(freqs_sin, dtype=np.float32)
    d = np.arange(128)
    pair = (d % 64) // 2
    cosd = np.ascontiguousarray(cos[:, pair].T).astype(bf)  # [128, S]
    sgn = np.where(d % 2 == 0, -1.0, 1.0).astype(np.float32)
    sind = np.ascontiguousarray(sin[:, pair].T * sgn[:, None]).astype(bf)

    perm = np.zeros((128, 128), dtype=np.float32)
    idx = np.arange(128)
    perm[idx ^ 1, idx] = 1.0
    perm = perm.astype(bf)
    eye128 = np.eye(128, dtype=np.float32).astype(bf)

    MW = max(total_w, 128)
    mm = np.zeros((128, MW), dtype=np.float32)
    off = 0
    for blk in blocks:
        mm[:, off : off + blk.shape[1]] = blk
        off += blk.shape[1]
    maskmix = np.ascontiguousarray(mm).astype(bf)

    wq = np.asarray(wq, dtype=np.float32) * 0.125  # fold 1/sqrt(HEAD_DIM)
    wk = np.asarray(wk, dtype=np.float32)
    wv = np.asarray(wv, dtype=np.float32)
    wo = np.asarray(wo, dtype=np.float32)

    # host-tile x per batch: [DIM, S] -> [p, nt, kt, tok'] flat
    xts = []
    for b in range(B):
        xT = x32[b].T  # [DIM, S]
        xt = (
            xT.reshape(NKT, 128, NQC, 512)
            .transpose(1, 2, 0, 3)
            .reshape(128, NQC * NKT * 512)
        )
        xts.append(np.ascontiguousarray(xt).astype(bf))

    in_maps = []
    for c in range(N_CORES):
        b, t = divmod(c, TP)
        hs = slice(t * HPC * HEAD_DIM, (t + 1) * HPC * HEAD_DIM)
        kvparts = []
        for j in range(KVPC):
            ks = slice((KVPC * t + j) * HEAD_DIM, (KVPC * t + j + 1) * HEAD_DIM)
            kvparts.append(wk[:, ks])
            kvparts.append(wv[:, ks])
        wkv = np.concatenate(kvparts, axis=1)  # [DIM, 256]
        # host-tile weights to [p, kt, m] (contiguous per-kt chunks)
        wqt = (
            wq[:, hs]
            .reshape(NKT, 128, HPC * HEAD_DIM)
            .transpose(1, 0, 2)
            .reshape(128, NKT * HPC * HEAD_DIM)
        )
        wkvt = (
            wkv.reshape(NKT, 128, KVPC * 2 * HEAD_DIM)
            .transpose(1, 0, 2)
            .reshape(128, NKT * KVPC * 2 * HEAD_DIM)
        )
        in_maps.append(
            {
                "xt_c": xts[b],
                "wq_c": np.ascontiguousarray(wqt).astype(bf),
                "wkv_c": np.ascontiguousarray(wkvt).astype(bf),
                "wo_c": np.ascontiguousarray(wo[hs, :]).astype(bf),
                "maskmix": maskmix,
                "cosd": cosd,
                "sind": sind,
                "perm": perm,
                "eye128": eye128,
            }
        )
    return in_maps


def kernel(x, freqs_cos, freqs_sin, mask, wq, wk, wv, wo, _trace=False):
    tinfo, total_w, blocks = _analyze_mask(np.asarray(mask, dtype=np.float32))
    key = tuple(tuple(r) for r in tinfo)
    if key not in _NC_CACHE:
        _NC_CACHE[key] = build_nc(tinfo, total_w)
    nc = _NC_CACHE[key]
    in_maps = _prep_inputs(
        x, freqs_cos, freqs_sin, mask, wq, wk, wv, wo, blocks, total_w
    )
    res = run_bass_kernel_spmd(
        nc, in_maps, core_ids=list(range(N_CORES)), trace=_trace
    )
    out = np.zeros((B, S, DIM), dtype=np.float32)
    for c in range(N_CORES):
        b = c // TP
        out[b] += np.asarray(res.results[c]["out_c"], dtype=np.float32)
    if _trace:
        kernel._last_exec_time_ns = res.exec_time_ns
        kernel._last_profile_json = res.profile_json
    return out


# revision 5
# speedup vs baseline: 1.1259x; 1.1259x over previous
"""Self-contained Trainium2 Bass kernel for the GQA attention module (v2).

Sharding: 4-way head tensor-parallel x 2-way batch data-parallel.
Core c = 4*b + t owns batch b, q-heads [8t..8t+8), kv-heads {2t, 2t+1};
the host sums the 4 TP partials per batch (the "all-reduce after wo").

v2 design (vs the ~455-539us v1 8-way-TP baseline):
  - Projections and attention are interleaved per 512-token block: project
    block nt, then run the two attention units (one per kv head) for
    q-chunk nt.  The PE-dense projection matmuls fill the ACT-heavy
    attention stretches, so the PE never idles long enough for the HAM
    clock gate to re-throttle it to half rate.
  - Causal column restriction: per (kt, qc) score tile only the q columns
    with any unmasked row are computed (scores, exp, PV all restricted).
  - The additive mask is applied on the PE: an identity-weighted matmul
    accumulates the raw mask block into the score PSUM before exp, so exp
    writes straight to the P^T ring and the DVE mask multiplies are gone.
  - The PV accumulator (pso) is copied to SBUF right after the last PV
    matmul, releasing its 4 PSUM banks early so the next unit's PV is
    never blocked (pso pool has bufs=1; all other PSUM transients share
    one 2-deep [128,1024] ring).
  - wo runs per q-chunk (needs both kv-head units' at); its matmuls and
    the normalize chain are deferred closures drained at a paced rate
    during the following projection/attention slots, as in v1.
"""

import sys
import types

sys.path.insert(0, "/opt/trn_rl_repo")

import numpy as np
import ml_dtypes


def _install_axon_hook_shim():
    import antenv

    if "antenv.axon_hooks" in sys.modules:
        return
    m = types.ModuleType("antenv.axon_hooks")
    m._hook = None

    def set_axon_ntff_profile_hook(h):
        m._hook = h

    def get_axon_ntff_profile_hook():
        return m._hook

    m.set_axon_ntff_profile_hook = set_axon_ntff_profile_hook
    m.get_axon_ntff_profile_hook = get_axon_ntff_profile_hook
    sys.modules["antenv.axon_hooks"] = m
    antenv.axon_hooks = m
    try:
        from trn_agent_boot.trn_boot import _ntff_profile_via_ctypes

        hook = _ntff_profile_via_ctypes("/opt/axon/libaxon_pjrt.so")
        if hook is not None:
            m.set_axon_ntff_profile_hook(hook)
    except Exception:
        pass


_install_axon_hook_shim()

import concourse.bass as bass
import concourse.mybir as mybir
import concourse.tile as tile
from concourse.bass_utils import run_bass_kernel_spmd

BF16 = mybir.dt.bfloat16
F32 = mybir.dt.float32

B, S, DIM = 2, 2048, 2048
N_HEADS, N_KV_HEADS, HEAD_DIM = 32, 8, 64
N_CORES = 8
TP = 4  # head-parallel groups
HPC = N_HEADS // TP  # 8 q heads per core
KVPC = N_KV_HEADS // TP  # 2 kv heads per core
NPAIR = HPC // 2  # 4 head pairs per core
TOK = S  # tokens per core (its batch)
NKT = S // 128  # 16 k tiles
NQC = S // 512  # 4 q chunks
NEG_THRESH = -1e4


def _patched_drain_and_barrier(self, tick_clock, wait_clock):
    # walrus (CoreV3) only accepts one sync-wait on the tile exit drain;
    # split the accumulated waits across single-wait nops.
    nc = self.nc
    drain_inst = nc.sync.drain()
    wait_clock.add_sem_waits(
        drain_inst.ins, tile.ScopedClock({None: tick_clock.global_clock})
    )
    si = drain_inst.ins.sync_info
    sw = list(si.on_wait) if si and si.on_wait else []
    if len(sw) > 1:
        si.on_wait = [sw[0]]
        for w in sw[1:]:
            n2 = nc.sync.nop(nofuse=True)
            if n2.ins.sync_info is None:
                n2.ins.sync_info = mybir.SyncInfo(on_wait=[w], on_update=[])
            else:
                n2.ins.sync_info.on_wait = [w]
    nc.all_engine_barrier()
    assert self.sems is not None
    popped = nc._tile_sem_poison_stack.pop()
    assert popped is self._sem_poison
    nc.clear_and_free_semaphores(list(self.sems.allocated().values()))
    nc.all_engine_barrier()


tile.TileContext._drain_and_barrier = _patched_drain_and_barrier


def _split_multi_waits(nc):
    """walrus (this build) accepts at most one sync-wait per instruction;
    move extra waits onto same-engine nops inserted just before."""
    n_split = 0
    for f in nc.m.functions:
        for blk in f.blocks:
            new_insts = []
            for inst in blk.instructions:
                si = getattr(inst, "sync_info", None)
                if si is not None and si.on_wait and len(si.on_wait) > 1:
                    extra = list(si.on_wait[:-1])
                    si.on_wait = [si.on_wait[-1]]
                    for w in extra:
                        nop = mybir.InstNoOp(
                            name=nc.get_next_instruction_name(), ins=[], outs=[]
                        )
                        nop.engine = inst.engine
                        nop.sync_info = mybir.SyncInfo(on_wait=[w], on_update=[])
                        new_insts.append(nop)
                        n_split += 1
                new_insts.append(inst)
            blk.instructions[:] = new_insts
    return n_split


def build_nc(tinfo, mask_total_w):
    """tinfo[kt][qc] = ('n'|'t', c0, c1, moff): fully-masked / take, with
    computed q-col range [c0, 512), mixed mask cols [c0, c1) at maskmix
    offset moff (c1 == c0 means no mask needed)."""
    nc = bass.Bass("TRN2", target_bir_lowering=False, debug=False, num_devices=N_CORES)

    MW = max(mask_total_w, 128)
    # x is host-tiled to [p, nt, kt, tok'] so each 512-token block is one
    # contiguous-per-partition 16 KiB-line DMA
    xt_d = nc.dram_tensor("xt_c", [128, NQC * NKT * 512], BF16, kind="ExternalInput")
    # wq/wkv are host-tiled to [p, kt, m] so per-kt chunks are contiguous
    wq_d = nc.dram_tensor(
        "wq_c", [128, NKT * HPC * HEAD_DIM], BF16, kind="ExternalInput"
    )
    wkv_d = nc.dram_tensor(
        "wkv_c", [128, NKT * KVPC * 2 * HEAD_DIM], BF16, kind="ExternalInput"
    )
    wo_d = nc.dram_tensor("wo_c", [HPC * HEAD_DIM, DIM], BF16, kind="ExternalInput")
    maskmix_d = nc.dram_tensor("maskmix", [128, MW], BF16, kind="ExternalInput")
    cosd_d = nc.dram_tensor("cosd", [128, TOK], BF16, kind="ExternalInput")
    sind_d = nc.dram_tensor("sind", [128, TOK], BF16, kind="ExternalInput")
    perm_d = nc.dram_tensor("perm", [128, 128], BF16, kind="ExternalInput")
    eye128_d = nc.dram_tensor("eye128", [128, 128], BF16, kind="ExternalInput")
    out_d = nc.dram_tensor("out_c", [TOK, DIM], BF16, kind="ExternalOutput")

    with tile.TileContext(nc) as tc:
        with (
            tc.tile_pool(name="persist", bufs=1) as persist,
            tc.tile_pool(name="stream", bufs=2) as stream,
            tc.tile_pool(name="small", bufs=2) as small,
            tc.tile_pool(name="otp", bufs=2) as otp,
            tc.tile_pool(name="bcp", bufs=2) as bcp,
        ):
            # ---- persistent tensors ----
            wq_sb = persist.tile([128, NKT, HPC * HEAD_DIM], BF16, tag="wq")
            wkv_sb = persist.tile([128, NKT, KVPC * 2 * HEAD_DIM], BF16, tag="wkv")
            wo_sb = persist.tile([128, NPAIR, DIM], BF16, tag="wo")
            perm_sb = persist.tile([128, 128], BF16, tag="perm")
            eye_sb = persist.tile([128, 128], BF16, tag="eye")
            cos_sb = persist.tile([128, TOK], BF16, tag="cos")
            sin_sb = persist.tile([128, TOK], BF16, tag="sin")
            mask_sb = persist.tile([128, MW], BF16, tag="mask")
            q_sb = persist.tile([128, NPAIR, TOK], BF16, tag="q")  # Q^T
            kT_sb = persist.tile([128, KVPC, TOK], BF16, tag="kT")  # K^T dup halves
            v_sb = persist.tile([128, KVPC * NKT, 68], BF16, tag="v")  # [V|1|pad]
            at_sb = persist.tile([128, NPAIR, TOK], BF16, tag="at")  # normalized A^T
            # raw A^T ring (row 64 carries the softmax denominators)
            aout_sb = persist.tile([65, 2, 4, 512], BF16, tag="aout")
            rec32_sb = persist.tile([65, 2048], F32, tag="rec32")  # ln d (row 64)
            recb_sb = persist.tile([65, 2048], BF16, tag="recb")  # 1/d (row 64)
            ones_sb = persist.tile([128, 64], BF16, tag="ones")
            NRING = 4
            pring = persist.tile([128, NRING, 2048], BF16, tag="pring")

            # DMA emission in need-order: the first Q matmul needs xblk0[kt]
            # and the mt=0 slice of wq[kt], so those chunks go first.
            xblk0 = stream.tile([128, NKT, 512], BF16, tag="xblk")
            for g in range(4):
                nc.sync.dma_start(
                    xblk0[:, 4 * g : 4 * g + 4, :].rearrange("p t n -> p (t n)"),
                    xt_d[:, g * 2048 : (g + 1) * 2048],
                )
                # kv weights first: the kv tiles are projected first
                nc.sync.dma_start(
                    wkv_sb[:, 4 * g : 4 * g + 4, :].rearrange("p t n -> p (t n)"),
                    wkv_d[:, g * 1024 : (g + 1) * 1024],
                )
                nc.sync.dma_start(
                    wq_sb[:, 4 * g : 4 * g + 4, :].rearrange("p t n -> p (t n)"),
                    wq_d[:, g * 2048 : (g + 1) * 2048],
                )
            nc.sync.dma_start(perm_sb[:], perm_d[:])
            nc.sync.dma_start(eye_sb[:], eye128_d[:])
            nc.sync.dma_start(cos_sb[:], cosd_d[:])
            nc.sync.dma_start(sin_sb[:], sind_d[:])
            nc.sync.dma_start(mask_sb[:], maskmix_d[:])
            nc.sync.dma_start(wo_sb[:], wo_d.rearrange("(t p) m -> p t m", p=128))
            nc.gpsimd.memset(v_sb[:, :, 64:65], 1.0)
            nc.gpsimd.memset(ones_sb[:], 1.0)

            # PSUM: one 2-deep [128,1024] ring for every transient (score
            # pairs, projection accumulators, RoPE swaps, V-transpose, wo
            # chunks, 1/d broadcast) + a single 4-bank PV accumulator.
            ps_s_cm = tc.tile_pool(name="ps_s", bufs=2, space="PSUM")
            ps_s = ps_s_cm.__enter__()
            ps_o_cm = tc.tile_pool(name="ps_o", bufs=1, space="PSUM")
            ps_o = ps_o_cm.__enter__()

            # Two deferred-closure queues: light normalize work (pss-tag /
            # DVE) drains anywhere; wo chunks allocate the 4-bank pso slot
            # and may ONLY drain where pso is free (projection tile windows
            # and unit tails) — draining one mid-unit would deadlock the PE
            # queue behind the unit's own pso.
            pend_norm = []
            pend_wo = []  # entries: (need, closure) — need = norm ops that
            # must have drained first (the at_sb writes wo reads)
            norm_stat = {"queued": 0, "drained": 0}

            def drain(k):
                for _ in range(min(k, len(pend_norm))):
                    pend_norm.pop(0)()
                    norm_stat["drained"] += 1

            def drain_wo(k):
                for _ in range(min(k, len(pend_wo))):
                    need, op = pend_wo[0]
                    while norm_stat["drained"] < need and pend_norm:
                        pend_norm.pop(0)()
                        norm_stat["drained"] += 1
                    if norm_stat["drained"] < need:
                        return
                    pend_wo.pop(0)
                    op()

            # ---------------- projection of one 512-token block ----------
            def project_block(nt, xblk=None):
                cs = slice(nt * 512, (nt + 1) * 512)
                if xblk is None:
                    xblk = stream.tile([128, NKT, 512], BF16, tag="xblk")
                    base = nt * NKT * 512
                    for g in range(4):
                        nc.sync.dma_start(
                            xblk[:, 4 * g : 4 * g + 4, :].rearrange(
                                "p t n -> p (t n)"
                            ),
                            xt_d[:, base + g * 2048 : base + (g + 1) * 2048],
                        )
                cosb = cos_sb[:, cs]
                sinb = sin_sb[:, cs]

                # Projection tiles are software-pipelined: tile t's RoPE tail
                # (perm matmul + muls) is emitted after tile t+1's
                # accumulation matmuls, so the PE never queues behind the
                # PSUM->SBUF copy.  The pair-swap matmul writes into the
                # accumulator tile's unused second bank (no extra slot).
                def rope_q(psq, mt):
                    q_tmp = small.tile([128, 512], BF16, tag="q_tmp")
                    nc.scalar.copy(q_tmp[:], psq[:, 0:512])
                    nc.tensor.matmul(psq[:, 512:1024], perm_sb[:], q_tmp[:])
                    v1 = small.tile([128, 512], BF16, tag="v1")
                    nc.vector.tensor_mul(v1[:], q_tmp[:], cosb)
                    v2 = small.tile([128, 512], BF16, tag="v2")
                    nc.vector.tensor_mul(v2[:], psq[:, 512:1024], sinb)
                    nc.vector.tensor_add(q_sb[:, mt, cs], v1[:], v2[:])

                def rope_kv(pskv, j):
                    kv_tmp = small.tile([128, 512], BF16, tag="kv_tmp")
                    nc.scalar.copy(kv_tmp[:], pskv[:, 0:512])
                    # K RoPE on rows 0:64
                    nc.tensor.matmul(
                        pskv[0:64, 512:1024], perm_sb[0:64, 0:64], kv_tmp[0:64, :]
                    )
                    kv1 = small.tile([64, 512], BF16, tag="kv1")
                    nc.vector.tensor_mul(kv1[:], kv_tmp[0:64, :], cosb[0:64, :])
                    kv2 = small.tile([64, 512], BF16, tag="kv2")
                    nc.vector.tensor_mul(kv2[:], pskv[0:64, 512:1024], sinb[0:64, :])
                    nc.vector.tensor_add(kT_sb[0:64, j, cs], kv1[:], kv2[:])
                    # duplicate K^T into partitions 64..127 (so the row-packed
                    # score matmul pairs get distinct PE row groups)
                    nc.gpsimd.dma_start(kT_sb[64:128, j, cs], kT_sb[0:64, j, cs])

                    # V transpose: rows 64:128 of kv_tmp -> natural V [k, 64]
                    pst = ps_s.tile([128, 4, 64], BF16, tag="pss", name="pst")
                    for jq in range(4):
                        nc.tensor.transpose(
                            pst[:, jq, :],
                            kv_tmp[64:128, jq * 128 : (jq + 1) * 128],
                            eye_sb[64:128, 64:128],
                            tile_position=(64, 0),
                        )
                    rc0 = j * NKT + nt * 4
                    nc.scalar.copy(v_sb[:, rc0 : rc0 + 4, 0:64], pst[:])

                st = {"tail": None}

                def do_tiles(tiles):
                    for kind, idx in tiles:
                        ps = ps_s.tile([128, 1024], F32, tag="pss", name="psp")
                        wsb = wq_sb if kind == "q" else wkv_sb
                        for kt in range(NKT):
                            nc.tensor.matmul(
                                ps[:, 0:512],
                                wsb[:, kt, idx * 128 : (idx + 1) * 128],
                                xblk[:, kt, :],
                                start=(kt == 0),
                                stop=(kt == NKT - 1),
                            )
                        if st["tail"]:
                            st["tail"]()
                        if kind == "q":
                            st["tail"] = lambda ps=ps, idx=idx: rope_q(ps, idx)
                        else:
                            st["tail"] = lambda ps=ps, idx=idx: rope_kv(ps, idx)
                        drain(1)
                        drain_wo(1)

                def flush():
                    if st["tail"]:
                        st["tail"]()
                        st["tail"] = None

                return do_tiles, flush

            # ---------------- one attention unit: (qc, kv head j) --------
            def attn_unit(qc, j, uidx, wo_slots=0):
                acts = [kt for kt in range(NKT) if tinfo[kt][qc][0] != "n"]
                assert acts, "fully-masked q chunk unsupported"
                ring = aout_sb[:, uidx % 2]  # [64, 4, 512]
                u = {"qc": qc, "j": j, "ring": ring, "pso": None}
                # wo chunks on the pso slot may drain here only when this
                # unit follows a projection block (the previous unit's pso
                # has long been released, so the PE queue won't block)
                drain_wo(wo_slots)

                def emit_pv(i):
                    kt = acts[i]
                    c0 = tinfo[kt][qc][1]
                    rc = j * NKT + kt
                    for h in range(4):
                        nc.tensor.matmul(
                            u["pso"][:, h, c0:512],
                            v_sb[:, rc, 0:65],
                            pring[:, i % NRING, h * 512 + c0 : (h + 1) * 512],
                            start=(i == 0),
                            stop=(i == len(acts) - 1),
                        )

                for i, kt in enumerate(acts):
                    cls, c0, c1, moff = tinfo[kt][qc]
                    if i == 0:
                        assert c0 == 0, "first active kt must be full-width"
                    ks = slice(kt * 128, (kt + 1) * 128)
                    mw = c1 - c0
                    # scores for 4 heads: 2 row-group-packed matmul pairs
                    for pair in range(2):
                        mt = 2 * j + pair
                        qs = slice(qc * 512 + c0, (qc + 1) * 512)
                        pss = ps_s.tile([128, 1024], F32, tag="pss", name="pss")
                        nc.tensor.matmul(
                            pss[:, c0:512],
                            kT_sb[0:64, j, ks],
                            q_sb[0:64, mt, qs],
                            tile_position=(0, 0),
                            start=True,
                            stop=(mw == 0),
                        )
                        nc.tensor.matmul(
                            pss[:, 512 + c0 : 1024],
                            kT_sb[64:128, j, ks],
                            q_sb[64:128, mt, qs],
                            tile_position=(64, 0),
                            start=True,
                            stop=(mw == 0),
                        )
                        if mw:
                            # accumulate the raw additive mask block into the
                            # mixed columns on the PE (identity stationary;
                            # two plain-AP matmuls — a broadcast moving
                            # operand here loses the RAW dep on the mask DMA)
                            nc.tensor.matmul(
                                pss[:, c0:c1],
                                eye_sb[:],
                                mask_sb[:, moff : moff + mw],
                                start=False,
                                stop=True,
                            )
                            nc.tensor.matmul(
                                pss[:, 512 + c0 : 512 + c1],
                                eye_sb[:],
                                mask_sb[:, moff : moff + mw],
                                start=False,
                                stop=True,
                            )
                        nc.scalar.activation(
                            pring[:, i % NRING, pair * 1024 : (pair + 1) * 1024]
                            .rearrange("p (t n) -> p t n", t=2)[:, :, c0:512],
                            pss[:].rearrange("p (t n) -> p t n", t=2)[:, :, c0:512],
                            mybir.ActivationFunctionType.Exp,
                        )
                    # drain light closures, paced across the sweep
                    k = (
                        -(-len(pend_norm) // max(1, len(acts) - i))
                        if pend_norm
                        else 0
                    )
                    drain(min(k, 2))
                    if i == 1:
                        u["pso"] = ps_o.tile([65, 4, 512], F32, tag="pso", name="pso")
                    if i >= 2:
                        emit_pv(i - 2)
                if len(acts) == 1:
                    u["pso"] = ps_o.tile([65, 4, 512], F32, tag="pso", name="pso")
                drain(len(pend_norm))
                for i in range(max(0, len(acts) - 2), len(acts)):
                    emit_pv(i)

                # release pso quickly: copy raw A^T plus the denominator row
                # (65 partitions) to SBUF, split across DVE and ACT.  The
                # ln/exp reciprocal chain runs later as deferred closures so
                # it never delays the next unit's exps in the ACT FIFO.
                pso = u["pso"]
                nc.vector.tensor_copy(ring[:, :, :], pso[:, :, :])
                return u

            def norm_ops(u):
                """Deferred normalize closures for unit u (reads aout ring)."""
                qc, j, ring = u["qc"], u["j"], u["ring"]
                qcs = slice(qc * 512, (qc + 1) * 512)
                ops = []

                def op_ln():
                    nc.scalar.activation(
                        rec32_sb[64:65, :],
                        ring[64:65, :, :].rearrange("p a n -> p (a n)"),
                        mybir.ActivationFunctionType.Ln,
                    )

                def op_recip():
                    nc.scalar.activation(
                        recb_sb[64:65, :],
                        rec32_sb[64:65, :],
                        mybir.ActivationFunctionType.Exp,
                        scale=-1.0,
                    )

                ops.append(op_ln)
                ops.append(op_recip)
                psbs = [None, None]
                for p in range(2):  # head pair within this kv head
                    ch = 2 * j + p

                    def op_psb(p=p):
                        # partition-broadcast 1/d to 64 rows via a K=1 matmul
                        psb = ps_s.tile([128, 1024], F32, tag="pss", name="psb")
                        psbs[p] = psb
                        for hh in range(2):
                            h = 2 * p + hh
                            nc.tensor.matmul(
                                psb[0:64, hh * 512 : (hh + 1) * 512],
                                ones_sb[64:65, :],
                                recb_sb[64:65, h * 512 : (h + 1) * 512],
                            )

                    def op_mul(p=p, ch=ch):
                        bc = bcp.tile([64, 1024], BF16, tag="bc")
                        nc.vector.tensor_copy(bc[:], psbs[p][0:64, :])
                        nc.vector.tensor_mul(
                            at_sb[0:64, ch, qcs], ring[0:64, 2 * p, :], bc[:, 0:512]
                        )
                        att = small.tile([64, 512], BF16, tag="att")
                        nc.vector.tensor_mul(
                            att[:], ring[0:64, 2 * p + 1, :], bc[:, 512:1024]
                        )
                        nc.gpsimd.dma_start(at_sb[64:128, ch, qcs], att[:])

                    ops.append(op_psb)
                    ops.append(op_mul)
                return ops

            def wo_ops(qc, pool, tag, act_cast=True, jjs=range(4)):
                """wo chunk closures for q-chunk qc (needs both units' at)."""
                base = qc * 512
                ops = []
                for jj in jjs:
                    rs = slice(base + jj * 128, base + (jj + 1) * 128)
                    for half in range(2):

                        def op(rs=rs, half=half):
                            psd = pool.tile([128, 1024], F32, tag=tag, name="psd")
                            for sub in range(2):
                                ntc = half * 2 + sub
                                cs2 = slice(ntc * 512, (ntc + 1) * 512)
                                for ch in range(NPAIR):
                                    nc.tensor.matmul(
                                        psd[:, sub * 512 : (sub + 1) * 512],
                                        at_sb[:, ch, rs],
                                        wo_sb[:, ch, cs2],
                                        start=(ch == 0),
                                        stop=(ch == NPAIR - 1),
                                    )
                            ot = otp.tile([128, 1024], BF16, tag="ot")
                            # alternate the PSUM drain between DVE and ACT so
                            # two casts can be in flight (DVE-only when the
                            # chunks drain mid-attention, where ACT is busy)
                            if half == 1 and act_cast:
                                nc.scalar.copy(ot[:], psd[:])
                            else:
                                nc.vector.tensor_copy(ot[:], psd[:])
                            nc.sync.dma_start(
                                out_d[rs, half * 1024 : (half + 1) * 1024], ot[:]
                            )

                        ops.append(op)
                return ops

            # ---------------- main interleaved schedule ------------------
            # Projections run one block ahead of attention, split in halves
            # around the first attention unit of each chunk so every unit
            # start follows PE-dense projection work (keeps HAM warm and
            # lets the previous unit's exp backlog drain).  Early q-chunks'
            # wo uses the pso slot (drained in projection windows and at
            # post-projection unit starts); late chunks fall back to the
            # shared pss ring and drain mid-unit.
            HALF1 = [("q", 0), ("q", 1), ("q", 2)]
            HALF2 = [("q", 3), ("kv", 0), ("kv", 1)]
            uidx = 0
            do0, fl0 = project_block(0, xblk=xblk0)
            # kv tiles first for block 0: their weight tensor is smaller, so
            # the very first accumulation starts sooner after the DMAs
            do0([("kv", 0), ("kv", 1)] + HALF1[:3] + [("q", 3)])
            fl0()
            for qc in range(NQC):
                do = fl = None
                if qc + 1 < NQC:
                    do, fl = project_block(qc + 1)
                    do(HALF1)
                u = attn_unit(qc, 0, uidx, wo_slots=2)
                uidx += 1
                pend_norm.extend(norm_ops(u))
                norm_stat["queued"] += 6
                if do:
                    do(HALF2)
                    fl()
                u = attn_unit(qc, 1, uidx, wo_slots=(2 if do else 0))
                uidx += 1
                pend_norm.extend(norm_ops(u))
                norm_stat["queued"] += 6
                need = norm_stat["queued"]
                if qc < 2:
                    pend_wo.extend(
                        (need, op) for op in wo_ops(qc, ps_o, "pso")
                    )
                else:
                    # half on the pso slot (drains at the remaining unit
                    # starts / final drain), half on the shared ring so two
                    # psd streams can be in flight at the end
                    pend_wo.extend(
                        (need, op)
                        for op in wo_ops(qc, ps_o, "pso", jjs=range(0, 2))
                    )
                    extra = wo_ops(qc, ps_s, "pss", act_cast=False, jjs=range(2, 4))
                    pend_norm.extend(extra)
                    norm_stat["queued"] += len(extra)
            while pend_norm or pend_wo:
                drain(1)
                drain_wo(1)

            ps_o_cm.__exit__(None, None, None)
            ps_s_cm.__exit__(None, None, None)
    _split_multi_waits(nc)
    return nc


_NC_CACHE = {}


def _analyze_mask(mask):
    """Per (kt, qc) tile info from the [S, S] additive mask ([q, k]).

    Returns (tinfo, total_w, blocks): tinfo[kt][qc] = (cls, c0, c1, moff);
    blocks is the list of [128, w] transposed mask blocks to concatenate."""
    masked = mask <= NEG_THRESH  # [q, k] bool
    tinfo = [[None] * NQC for _ in range(NKT)]
    blocks = []
    moff = 0

    def add_block(q0, q1, kt):
        blk = np.asarray(
            mask[q0:q1, kt * 128 : (kt + 1) * 128].T, dtype=np.float32
        )
        blk = np.maximum(blk, -30000.0)
        blocks.append(blk)
        return blk.shape[1]

    for kt in range(NKT):
        for qc in range(NQC):
            sub = masked[qc * 512 : (qc + 1) * 512, kt * 128 : (kt + 1) * 128]
            col_all = sub.all(axis=1)  # q col fully masked
            col_any = sub.any(axis=1)
            if col_all.all():
                tinfo[kt][qc] = ("n", 0, 0, 0)
                continue
            c0 = int(np.argmin(col_all))  # first not-fully-masked col
            assert not col_all[c0:].any(), "non-contiguous masked col range"
            if col_any[c0:].any():
                c1 = 512 - int(np.argmax(col_any[::-1]))  # last any-masked col + 1
            else:
                c1 = c0
            if c1 > c0:
                w = add_block(qc * 512 + c0, qc * 512 + c1, kt)
                tinfo[kt][qc] = ("t", c0, c1, moff)
                moff += w
            else:
                tinfo[kt][qc] = ("t", c0, c0, 0)
    # the first active tile of each q chunk must be full width (it carries
    # the PSUM has_written init for scores and PV)
    for qc in range(NQC):
        acts = [kt for kt in range(NKT) if tinfo[kt][qc][0] != "n"]
        assert acts, "fully-masked q chunk unsupported"
        kt0 = acts[0]
        cls, c0, c1, _ = tinfo[kt0][qc]
        if c0 != 0:
            c1 = max(c1, c0)
            w = add_block(qc * 512, qc * 512 + c1, kt0)
            tinfo[kt0][qc] = ("t", 0, c1, moff)
            moff += w
    return tinfo, moff, blocks


def _prep_inputs(x, freqs_cos, freqs_sin, mask, wq, wk, wv, wo, blocks, total_w):
    bf = ml_dtypes.bfloat16
    x32 = np.asarray(x, dtype=np.float32)  # [B, S, DIM]

    cos = np.asarray(freqs_cos, dtype=np.float32)  # [S, 32]
    sin = np.asarray(freqs_sin, dtype=np.float32)
    d = np.arange(128)
    pair = (d % 64) // 2
    cosd = np.ascontiguousarray(cos[:, pair].T).astype(bf)  # [128, S]
    sgn = np.where(d % 2 == 0, -1.0, 1.0).astype(np.float32)
    sind = np.ascontiguousarray(sin[:, pair].T * sgn[:, None]).astype(bf)

    perm = np.zeros((128, 128), dtype=np.float32)
    idx = np.arange(128)
    perm[idx ^ 1, idx] = 1.0
    perm = perm.astype(bf)
    eye128 = np.eye(128, dtype=np.float32).astype(bf)

    MW = max(total_w, 128)
    mm = np.zeros((128, MW), dtype=np.float32)
    off = 0
    for blk in blocks:
        mm[:, off : off + blk.shape[1]] = blk
        off += blk.shape[1]
    maskmix = np.ascontiguousarray(mm).astype(bf)

    wq = np.asarray(wq, dtype=np.float32) * 0.125  # fold 1/sqrt(HEAD_DIM)
    wk = np.asarray(wk, dtype=np.float32)
    wv = np.asarray(wv, dtype=np.float32)
    wo = np.asarray(wo, dtype=np.float32)

    # host-tile x per batch: [DIM, S] -> [p, nt, kt, tok'] flat
    xts = []
    for b in range(B):
        xT = x32[b].T  # [DIM, S]
        xt = (
            xT.reshape(NKT, 128, NQC, 512)
            .transpose(1, 2, 0, 3)
            .reshape(128, NQC * NKT * 512)
        )
        xts.append(np.ascontiguousarray(xt).astype(bf))

    in_maps = []
    for c in range(N_CORES):
        b, t = divmod(c, TP)
        hs = slice(t * HPC * HEAD_DIM, (t + 1) * HPC * HEAD_DIM)
        kvparts = []
        for j in range(KVPC):
            ks = slice((KVPC * t + j) * HEAD_DIM, (KVPC * t + j + 1) * HEAD_DIM)
            kvparts.append(wk[:, ks])
            kvparts.append(wv[:, ks])
        wkv = np.concatenate(kvparts, axis=1)  # [DIM, 256]
        # host-tile weights to [p, kt, m] (contiguous per-kt chunks)
        wqt = (
            wq[:, hs]
            .reshape(NKT, 128, HPC * HEAD_DIM)
            .transpose(1, 0, 2)
            .reshape(128, NKT * HPC * HEAD_DIM)
        )
        wkvt = (
            wkv.reshape(NKT, 128, KVPC * 2 * HEAD_DIM)
            .transpose(1, 0, 2)
            .reshape(128, NKT * KVPC * 2 * HEAD_DIM)
        )
        in_maps.append(
            {
                "xt_c": xts[b],
                "wq_c": np.ascontiguousarray(wqt).astype(bf),
                "wkv_c": np.ascontiguousarray(wkvt).astype(bf),
                "wo_c": np.ascontiguousarray(wo[hs, :]).astype(bf),
                "maskmix": maskmix,
                "cosd": cosd,
                "sind": sind,
                "perm": perm,
                "eye128": eye128,
            }
        )
    return in_maps


def kernel(x, freqs_cos, freqs_sin, mask, wq, wk, wv, wo, _trace=False):
    tinfo, total_w, blocks = _analyze_mask(np.asarray(mask, dtype=np.float32))
    key = tuple(tuple(r) for r in tinfo)
    if key not in _NC_CACHE:
        _NC_CACHE[key] = build_nc(tinfo, total_w)
    nc = _NC_CACHE[key]
    in_maps = _prep_inputs(
        x, freqs_cos, freqs_sin, mask, wq, wk, wv, wo, blocks, total_w
    )
    res = run_bass_kernel_spmd(
        nc, in_maps, core_ids=list(range(N_CORES)), trace=_trace
    )
    out = np.zeros((B, S, DIM), dtype=np.float32)
    for c in range(N_CORES):
        b = c // TP
        out[b] += np.asarray(res.results[c]["out_c"], dtype=np.float32)
    if _trace:
        kernel._last_exec_time_ns = res.exec_time_ns
        kernel._last_profile_json = res.profile_json
    return out


# revision 6
# speedup vs baseline: 1.1339x; 1.0072x over previous
"""Self-contained Trainium2 Bass kernel for the GQA attention module (v2).

Sharding: 4-way head tensor-parallel x 2-way batch data-parallel.
Core c = 4*b + t owns batch b, q-heads [8t..8t+8), kv-heads {2t, 2t+1};
the host sums the 4 TP partials per batch (the "all-reduce after wo").

v2 design (vs the ~455-539us v1 8-way-TP baseline):
  - Projections and attention are interleaved per 512-token block: project
    block nt, then run the two attention units (one per kv head) for
    q-chunk nt.  The PE-dense projection matmuls fill the ACT-heavy
    attention stretches, so the PE never idles long enough for the HAM
    clock gate to re-throttle it to half rate.
  - Causal column restriction: per (kt, qc) score tile only the q columns
    with any unmasked row are computed (scores, exp, PV all restricted).
  - The additive mask is applied on the PE: an identity-weighted matmul
    accumulates the raw mask block into the score PSUM before exp, so exp
    writes straight to the P^T ring and the DVE mask multiplies are gone.
  - The PV accumulator (pso) is copied to SBUF right after the last PV
    matmul, releasing its 4 PSUM banks early so the next unit's PV is
    never blocked (pso pool has bufs=1; all other PSUM transients share
    one 2-deep [128,1024] ring).
  - wo runs per q-chunk (needs both kv-head units' at); its matmuls and
    the normalize chain are deferred closures drained at a paced rate
    during the following projection/attention slots, as in v1.
"""

import sys
import types

sys.path.insert(0, "/opt/trn_rl_repo")

import numpy as np
import ml_dtypes


def _install_axon_hook_shim():
    import antenv

    if "antenv.axon_hooks" in sys.modules:
        return
    m = types.ModuleType("antenv.axon_hooks")
    m._hook = None

    def set_axon_ntff_profile_hook(h):
        m._hook = h

    def get_axon_ntff_profile_hook():
        return m._hook

    m.set_axon_ntff_profile_hook = set_axon_ntff_profile_hook
    m.get_axon_ntff_profile_hook = get_axon_ntff_profile_hook
    sys.modules["antenv.axon_hooks"] = m
    antenv.axon_hooks = m
    try:
        from trn_agent_boot.trn_boot import _ntff_profile_via_ctypes

        hook = _ntff_profile_via_ctypes("/opt/axon/libaxon_pjrt.so")
        if hook is not None:
            m.set_axon_ntff_profile_hook(hook)
    except Exception:
        pass


_install_axon_hook_shim()

import concourse.bass as bass
import concourse.mybir as mybir
import concourse.tile as tile
from concourse.bass_utils import run_bass_kernel_spmd

BF16 = mybir.dt.bfloat16
F32 = mybir.dt.float32

B, S, DIM = 2, 2048, 2048
N_HEADS, N_KV_HEADS, HEAD_DIM = 32, 8, 64
N_CORES = 8
TP = 4  # head-parallel groups
HPC = N_HEADS // TP  # 8 q heads per core
KVPC = N_KV_HEADS // TP  # 2 kv heads per core
NPAIR = HPC // 2  # 4 head pairs per core
TOK = S  # tokens per core (its batch)
NKT = S // 128  # 16 k tiles
NQC = S // 512  # 4 q chunks
NEG_THRESH = -1e4


def _patched_drain_and_barrier(self, tick_clock, wait_clock):
    # walrus (CoreV3) only accepts one sync-wait on the tile exit drain;
    # split the accumulated waits across single-wait nops.
    nc = self.nc
    drain_inst = nc.sync.drain()
    wait_clock.add_sem_waits(
        drain_inst.ins, tile.ScopedClock({None: tick_clock.global_clock})
    )
    si = drain_inst.ins.sync_info
    sw = list(si.on_wait) if si and si.on_wait else []
    if len(sw) > 1:
        si.on_wait = [sw[0]]
        for w in sw[1:]:
            n2 = nc.sync.nop(nofuse=True)
            if n2.ins.sync_info is None:
                n2.ins.sync_info = mybir.SyncInfo(on_wait=[w], on_update=[])
            else:
                n2.ins.sync_info.on_wait = [w]
    nc.all_engine_barrier()
    assert self.sems is not None
    popped = nc._tile_sem_poison_stack.pop()
    assert popped is self._sem_poison
    nc.clear_and_free_semaphores(list(self.sems.allocated().values()))
    nc.all_engine_barrier()


tile.TileContext._drain_and_barrier = _patched_drain_and_barrier


def _split_multi_waits(nc):
    """walrus (this build) accepts at most one sync-wait per instruction;
    move extra waits onto same-engine nops inserted just before."""
    n_split = 0
    for f in nc.m.functions:
        for blk in f.blocks:
            new_insts = []
            for inst in blk.instructions:
                si = getattr(inst, "sync_info", None)
                if si is not None and si.on_wait and len(si.on_wait) > 1:
                    extra = list(si.on_wait[:-1])
                    si.on_wait = [si.on_wait[-1]]
                    for w in extra:
                        nop = mybir.InstNoOp(
                            name=nc.get_next_instruction_name(), ins=[], outs=[]
                        )
                        nop.engine = inst.engine
                        nop.sync_info = mybir.SyncInfo(on_wait=[w], on_update=[])
                        new_insts.append(nop)
                        n_split += 1
                new_insts.append(inst)
            blk.instructions[:] = new_insts
    return n_split


def build_nc(tinfo, mask_total_w):
    """tinfo[kt][qc] = ('n'|'t', c0, c1, moff): fully-masked / take, with
    computed q-col range [c0, 512), mixed mask cols [c0, c1) at maskmix
    offset moff (c1 == c0 means no mask needed)."""
    nc = bass.Bass("TRN2", target_bir_lowering=False, debug=False, num_devices=N_CORES)

    MW = max(mask_total_w, 128)
    # x is host-tiled to [p, nt, kt, tok'] so each 512-token block is one
    # contiguous-per-partition 16 KiB-line DMA
    xt_d = nc.dram_tensor("xt_c", [128, NQC * NKT * 512], BF16, kind="ExternalInput")
    # wq/wkv are host-tiled to [p, kt, m] so per-kt chunks are contiguous
    wq_d = nc.dram_tensor(
        "wq_c", [128, NKT * HPC * HEAD_DIM], BF16, kind="ExternalInput"
    )
    wkv_d = nc.dram_tensor(
        "wkv_c", [128, NKT * KVPC * 2 * HEAD_DIM], BF16, kind="ExternalInput"
    )
    wo_d = nc.dram_tensor("wo_c", [HPC * HEAD_DIM, DIM], BF16, kind="ExternalInput")
    maskmix_d = nc.dram_tensor("maskmix", [128, MW], BF16, kind="ExternalInput")
    cosd_d = nc.dram_tensor("cosd", [128, TOK], BF16, kind="ExternalInput")
    sind_d = nc.dram_tensor("sind", [128, TOK], BF16, kind="ExternalInput")
    perm_d = nc.dram_tensor("perm", [128, 128], BF16, kind="ExternalInput")
    eye128_d = nc.dram_tensor("eye128", [128, 128], BF16, kind="ExternalInput")
    out_d = nc.dram_tensor("out_c", [TOK, DIM], BF16, kind="ExternalOutput")

    with tile.TileContext(nc) as tc:
        with (
            tc.tile_pool(name="persist", bufs=1) as persist,
            tc.tile_pool(name="stream", bufs=2) as stream,
            tc.tile_pool(name="small", bufs=2) as small,
            tc.tile_pool(name="otp", bufs=2) as otp,
            tc.tile_pool(name="bcp", bufs=2) as bcp,
        ):
            # ---- persistent tensors ----
            wq_sb = persist.tile([128, NKT, HPC * HEAD_DIM], BF16, tag="wq")
            wkv_sb = persist.tile([128, NKT, KVPC * 2 * HEAD_DIM], BF16, tag="wkv")
            wo_sb = persist.tile([128, NPAIR, DIM], BF16, tag="wo")
            perm_sb = persist.tile([128, 128], BF16, tag="perm")
            eye_sb = persist.tile([128, 128], BF16, tag="eye")
            cos_sb = persist.tile([128, TOK], BF16, tag="cos")
            sin_sb = persist.tile([128, TOK], BF16, tag="sin")
            mask_sb = persist.tile([128, MW], BF16, tag="mask")
            q_sb = persist.tile([128, NPAIR, TOK], BF16, tag="q")  # Q^T
            kT_sb = persist.tile([128, KVPC, TOK], BF16, tag="kT")  # K^T dup halves
            v_sb = persist.tile([128, KVPC * NKT, 68], BF16, tag="v")  # [V|1|pad]
            at_sb = persist.tile([128, NPAIR, TOK], BF16, tag="at")  # normalized A^T
            # raw A^T ring (row 64 carries the softmax denominators)
            aout_sb = persist.tile([65, 2, 4, 512], BF16, tag="aout")
            rec32_sb = persist.tile([65, 2048], F32, tag="rec32")  # ln d (row 64)
            recb_sb = persist.tile([65, 2048], BF16, tag="recb")  # 1/d (row 64)
            ones_sb = persist.tile([128, 64], BF16, tag="ones")
            NRING = 4
            pring = persist.tile([128, NRING, 2048], BF16, tag="pring")

            # DMA emission in need-order: the first Q matmul needs xblk0[kt]
            # and the mt=0 slice of wq[kt], so those chunks go first.
            xblk0 = stream.tile([128, NKT, 512], BF16, tag="xblk")
            for g in range(4):
                nc.sync.dma_start(
                    xblk0[:, 4 * g : 4 * g + 4, :].rearrange("p t n -> p (t n)"),
                    xt_d[:, g * 2048 : (g + 1) * 2048],
                )
                # kv weights first: the kv tiles are projected first
                nc.sync.dma_start(
                    wkv_sb[:, 4 * g : 4 * g + 4, :].rearrange("p t n -> p (t n)"),
                    wkv_d[:, g * 1024 : (g + 1) * 1024],
                )
                nc.sync.dma_start(
                    wq_sb[:, 4 * g : 4 * g + 4, :].rearrange("p t n -> p (t n)"),
                    wq_d[:, g * 2048 : (g + 1) * 2048],
                )
            nc.sync.dma_start(perm_sb[:], perm_d[:])
            nc.sync.dma_start(eye_sb[:], eye128_d[:])
            nc.sync.dma_start(cos_sb[:], cosd_d[:])
            nc.sync.dma_start(sin_sb[:], sind_d[:])
            nc.sync.dma_start(mask_sb[:], maskmix_d[:])
            nc.sync.dma_start(wo_sb[:], wo_d.rearrange("(t p) m -> p t m", p=128))
            nc.gpsimd.memset(v_sb[:, :, 64:65], 1.0)
            nc.gpsimd.memset(ones_sb[:], 1.0)

            # PSUM: one 2-deep [128,1024] ring for every transient (score
            # pairs, projection accumulators, RoPE swaps, V-transpose, wo
            # chunks, 1/d broadcast) + a single 4-bank PV accumulator.
            ps_s_cm = tc.tile_pool(name="ps_s", bufs=2, space="PSUM")
            ps_s = ps_s_cm.__enter__()
            ps_o_cm = tc.tile_pool(name="ps_o", bufs=1, space="PSUM")
            ps_o = ps_o_cm.__enter__()

            # Two deferred-closure queues: light normalize work (pss-tag /
            # DVE) drains anywhere; wo chunks allocate the 4-bank pso slot
            # and may ONLY drain where pso is free (projection tile windows
            # and unit tails) — draining one mid-unit would deadlock the PE
            # queue behind the unit's own pso.
            pend_norm = []
            pend_wo = []  # entries: (need, closure) — need = norm ops that
            # must have drained first (the at_sb writes wo reads)
            norm_stat = {"queued": 0, "drained": 0}

            def drain(k):
                for _ in range(min(k, len(pend_norm))):
                    pend_norm.pop(0)()
                    norm_stat["drained"] += 1

            def drain_wo(k):
                for _ in range(min(k, len(pend_wo))):
                    need, op = pend_wo[0]
                    while norm_stat["drained"] < need and pend_norm:
                        pend_norm.pop(0)()
                        norm_stat["drained"] += 1
                    if norm_stat["drained"] < need:
                        return
                    pend_wo.pop(0)
                    op()

            # ---------------- projection of one 512-token block ----------
            def project_block(nt, xblk=None):
                cs = slice(nt * 512, (nt + 1) * 512)
                if xblk is None:
                    xblk = stream.tile([128, NKT, 512], BF16, tag="xblk")
                    base = nt * NKT * 512
                    for g in range(4):
                        nc.sync.dma_start(
                            xblk[:, 4 * g : 4 * g + 4, :].rearrange(
                                "p t n -> p (t n)"
                            ),
                            xt_d[:, base + g * 2048 : base + (g + 1) * 2048],
                        )
                cosb = cos_sb[:, cs]
                sinb = sin_sb[:, cs]

                # Projection tiles are software-pipelined: tile t's RoPE tail
                # (perm matmul + muls) is emitted after tile t+1's
                # accumulation matmuls, so the PE never queues behind the
                # PSUM->SBUF copy.  The pair-swap matmul writes into the
                # accumulator tile's unused second bank (no extra slot).
                def rope_q(psq, mt):
                    q_tmp = small.tile([128, 512], BF16, tag="q_tmp")
                    nc.scalar.copy(q_tmp[:], psq[:, 0:512])
                    nc.tensor.matmul(psq[:, 512:1024], perm_sb[:], q_tmp[:])
                    v1 = small.tile([128, 512], BF16, tag="v1")
                    nc.vector.tensor_mul(v1[:], q_tmp[:], cosb)
                    v2 = small.tile([128, 512], BF16, tag="v2")
                    nc.vector.tensor_mul(v2[:], psq[:, 512:1024], sinb)
                    nc.vector.tensor_add(q_sb[:, mt, cs], v1[:], v2[:])

                def rope_kv(pskv, j):
                    kv_tmp = small.tile([128, 512], BF16, tag="kv_tmp")
                    nc.scalar.copy(kv_tmp[:], pskv[:, 0:512])
                    # K RoPE on rows 0:64
                    nc.tensor.matmul(
                        pskv[0:64, 512:1024], perm_sb[0:64, 0:64], kv_tmp[0:64, :]
                    )
                    kv1 = small.tile([64, 512], BF16, tag="kv1")
                    nc.vector.tensor_mul(kv1[:], kv_tmp[0:64, :], cosb[0:64, :])
                    kv2 = small.tile([64, 512], BF16, tag="kv2")
                    nc.vector.tensor_mul(kv2[:], pskv[0:64, 512:1024], sinb[0:64, :])
                    nc.vector.tensor_add(kT_sb[0:64, j, cs], kv1[:], kv2[:])
                    # duplicate K^T into partitions 64..127 (so the row-packed
                    # score matmul pairs get distinct PE row groups)
                    nc.gpsimd.dma_start(kT_sb[64:128, j, cs], kT_sb[0:64, j, cs])

                    # V transpose: rows 64:128 of kv_tmp -> natural V [k, 64]
                    pst = ps_s.tile([128, 4, 64], BF16, tag="pss", name="pst")
                    for jq in range(4):
                        nc.tensor.transpose(
                            pst[:, jq, :],
                            kv_tmp[64:128, jq * 128 : (jq + 1) * 128],
                            eye_sb[64:128, 64:128],
                            tile_position=(64, 0),
                        )
                    rc0 = j * NKT + nt * 4
                    nc.scalar.copy(v_sb[:, rc0 : rc0 + 4, 0:64], pst[:])

                st = {"tail": None}

                def do_tiles(tiles):
                    for kind, idx in tiles:
                        ps = ps_s.tile([128, 1024], F32, tag="pss", name="psp")
                        wsb = wq_sb if kind == "q" else wkv_sb
                        for kt in range(NKT):
                            nc.tensor.matmul(
                                ps[:, 0:512],
                                wsb[:, kt, idx * 128 : (idx + 1) * 128],
                                xblk[:, kt, :],
                                start=(kt == 0),
                                stop=(kt == NKT - 1),
                            )
                        if st["tail"]:
                            st["tail"]()
                        if kind == "q":
                            st["tail"] = lambda ps=ps, idx=idx: rope_q(ps, idx)
                        else:
                            st["tail"] = lambda ps=ps, idx=idx: rope_kv(ps, idx)
                        drain(2)
                        drain_wo(1)

                def flush():
                    if st["tail"]:
                        st["tail"]()
                        st["tail"] = None

                return do_tiles, flush

            # ---------------- one attention unit: (qc, kv head j) --------
            def attn_unit(qc, j, uidx, wo_slots=0):
                acts = [kt for kt in range(NKT) if tinfo[kt][qc][0] != "n"]
                assert acts, "fully-masked q chunk unsupported"
                ring = aout_sb[:, uidx % 2]  # [64, 4, 512]
                u = {"qc": qc, "j": j, "ring": ring, "pso": None}
                # wo chunks on the pso slot may drain here only when this
                # unit follows a projection block (the previous unit's pso
                # has long been released, so the PE queue won't block)
                drain_wo(wo_slots)

                def emit_pv(i):
                    kt = acts[i]
                    c0 = tinfo[kt][qc][1]
                    rc = j * NKT + kt
                    for h in range(4):
                        nc.tensor.matmul(
                            u["pso"][:, h, c0:512],
                            v_sb[:, rc, 0:65],
                            pring[:, i % NRING, h * 512 + c0 : (h + 1) * 512],
                            start=(i == 0),
                            stop=(i == len(acts) - 1),
                        )

                for i, kt in enumerate(acts):
                    cls, c0, c1, moff = tinfo[kt][qc]
                    if i == 0:
                        assert c0 == 0, "first active kt must be full-width"
                    ks = slice(kt * 128, (kt + 1) * 128)
                    mw = c1 - c0
                    # scores for 4 heads: 2 row-group-packed matmul pairs
                    for pair in range(2):
                        mt = 2 * j + pair
                        qs = slice(qc * 512 + c0, (qc + 1) * 512)
                        pss = ps_s.tile([128, 1024], F32, tag="pss", name="pss")
                        nc.tensor.matmul(
                            pss[:, c0:512],
                            kT_sb[0:64, j, ks],
                            q_sb[0:64, mt, qs],
                            tile_position=(0, 0),
                            start=True,
                            stop=(mw == 0),
                        )
                        nc.tensor.matmul(
                            pss[:, 512 + c0 : 1024],
                            kT_sb[64:128, j, ks],
                            q_sb[64:128, mt, qs],
                            tile_position=(64, 0),
                            start=True,
                            stop=(mw == 0),
                        )
                        if mw:
                            # accumulate the raw additive mask block into the
                            # mixed columns on the PE (identity stationary;
                            # two plain-AP matmuls — a broadcast moving
                            # operand here loses the RAW dep on the mask DMA)
                            nc.tensor.matmul(
                                pss[:, c0:c1],
                                eye_sb[:],
                                mask_sb[:, moff : moff + mw],
                                start=False,
                                stop=True,
                            )
                            nc.tensor.matmul(
                                pss[:, 512 + c0 : 512 + c1],
                                eye_sb[:],
                                mask_sb[:, moff : moff + mw],
                                start=False,
                                stop=True,
                            )
                        nc.scalar.activation(
                            pring[:, i % NRING, pair * 1024 : (pair + 1) * 1024]
                            .rearrange("p (t n) -> p t n", t=2)[:, :, c0:512],
                            pss[:].rearrange("p (t n) -> p t n", t=2)[:, :, c0:512],
                            mybir.ActivationFunctionType.Exp,
                        )
                    # drain light closures, paced across the sweep
                    k = (
                        -(-len(pend_norm) // max(1, len(acts) - i))
                        if pend_norm
                        else 0
                    )
                    drain(min(k, 2))
                    if i == 1:
                        u["pso"] = ps_o.tile([65, 4, 512], F32, tag="pso", name="pso")
                    if i >= 2:
                        emit_pv(i - 2)
                if len(acts) == 1:
                    u["pso"] = ps_o.tile([65, 4, 512], F32, tag="pso", name="pso")
                drain(len(pend_norm))
                for i in range(max(0, len(acts) - 2), len(acts)):
                    emit_pv(i)

                # release pso quickly: copy raw A^T plus the denominator row
                # (65 partitions) to SBUF, split across DVE and ACT.  The
                # ln/exp reciprocal chain runs later as deferred closures so
                # it never delays the next unit's exps in the ACT FIFO.
                pso = u["pso"]
                nc.vector.tensor_copy(ring[:, :, :], pso[:, :, :])
                return u

            def norm_ops(u):
                """Deferred normalize closures for unit u (reads aout ring)."""
                qc, j, ring = u["qc"], u["j"], u["ring"]
                qcs = slice(qc * 512, (qc + 1) * 512)
                ops = []

                def op_ln():
                    nc.scalar.activation(
                        rec32_sb[64:65, :],
                        ring[64:65, :, :].rearrange("p a n -> p (a n)"),
                        mybir.ActivationFunctionType.Ln,
                    )

                def op_recip():
                    nc.scalar.activation(
                        recb_sb[64:65, :],
                        rec32_sb[64:65, :],
                        mybir.ActivationFunctionType.Exp,
                        scale=-1.0,
                    )

                ops.append(op_ln)
                ops.append(op_recip)
                psbs = [None, None]
                for p in range(2):  # head pair within this kv head
                    ch = 2 * j + p

                    def op_psb(p=p):
                        # partition-broadcast 1/d to 64 rows via a K=1 matmul
                        psb = ps_s.tile([128, 1024], F32, tag="pss", name="psb")
                        psbs[p] = psb
                        for hh in range(2):
                            h = 2 * p + hh
                            nc.tensor.matmul(
                                psb[0:64, hh * 512 : (hh + 1) * 512],
                                ones_sb[64:65, :],
                                recb_sb[64:65, h * 512 : (h + 1) * 512],
                            )

                    def op_mul(p=p, ch=ch):
                        bc = bcp.tile([64, 1024], BF16, tag="bc")
                        nc.vector.tensor_copy(bc[:], psbs[p][0:64, :])
                        nc.vector.tensor_mul(
                            at_sb[0:64, ch, qcs], ring[0:64, 2 * p, :], bc[:, 0:512]
                        )
                        att = small.tile([64, 512], BF16, tag="att")
                        nc.vector.tensor_mul(
                            att[:], ring[0:64, 2 * p + 1, :], bc[:, 512:1024]
                        )
                        nc.gpsimd.dma_start(at_sb[64:128, ch, qcs], att[:])

                    ops.append(op_psb)
                    ops.append(op_mul)
                return ops

            def wo_ops(qc, pool, tag, act_cast=True, jjs=range(4)):
                """wo chunk closures for q-chunk qc (needs both units' at)."""
                base = qc * 512
                ops = []
                for jj in jjs:
                    rs = slice(base + jj * 128, base + (jj + 1) * 128)
                    for half in range(2):

                        def op(rs=rs, half=half):
                            psd = pool.tile([128, 1024], F32, tag=tag, name="psd")
                            for sub in range(2):
                                ntc = half * 2 + sub
                                cs2 = slice(ntc * 512, (ntc + 1) * 512)
                                for ch in range(NPAIR):
                                    nc.tensor.matmul(
                                        psd[:, sub * 512 : (sub + 1) * 512],
                                        at_sb[:, ch, rs],
                                        wo_sb[:, ch, cs2],
                                        start=(ch == 0),
                                        stop=(ch == NPAIR - 1),
                                    )
                            ot = otp.tile([128, 1024], BF16, tag="ot")
                            # alternate the PSUM drain between DVE and ACT so
                            # two casts can be in flight (DVE-only when the
                            # chunks drain mid-attention, where ACT is busy)
                            if half == 1 and act_cast:
                                nc.scalar.copy(ot[:], psd[:])
                            else:
                                nc.vector.tensor_copy(ot[:], psd[:])
                            nc.sync.dma_start(
                                out_d[rs, half * 1024 : (half + 1) * 1024], ot[:]
                            )

                        ops.append(op)
                return ops

            # ---------------- main interleaved schedule ------------------
            # Projections run one block ahead of attention, split in halves
            # around the first attention unit of each chunk so every unit
            # start follows PE-dense projection work (keeps HAM warm and
            # lets the previous unit's exp backlog drain).  Early q-chunks'
            # wo uses the pso slot (drained in projection windows and at
            # post-projection unit starts); late chunks fall back to the
            # shared pss ring and drain mid-unit.
            HALF1 = [("q", 0), ("q", 1), ("q", 2)]
            HALF2 = [("q", 3), ("kv", 0), ("kv", 1)]
            uidx = 0
            do0, fl0 = project_block(0, xblk=xblk0)
            # kv tiles first for block 0: their weight tensor is smaller, so
            # the very first accumulation starts sooner after the DMAs
            do0([("kv", 0), ("kv", 1)] + HALF1[:3] + [("q", 3)])
            fl0()
            for qc in range(NQC):
                do = fl = None
                if qc + 1 < NQC:
                    do, fl = project_block(qc + 1)
                    do(HALF1)
                u = attn_unit(qc, 0, uidx, wo_slots=2)
                uidx += 1
                pend_norm.extend(norm_ops(u))
                norm_stat["queued"] += 6
                if do:
                    do(HALF2)
                    fl()
                u = attn_unit(qc, 1, uidx, wo_slots=(2 if do else 0))
                uidx += 1
                pend_norm.extend(norm_ops(u))
                norm_stat["queued"] += 6
                need = norm_stat["queued"]
                if qc < 2:
                    pend_wo.extend(
                        (need, op) for op in wo_ops(qc, ps_o, "pso")
                    )
                else:
                    # half on the pso slot (drains at the remaining unit
                    # starts / final drain), half on the shared ring so two
                    # psd streams can be in flight at the end
                    pend_wo.extend(
                        (need, op)
                        for op in wo_ops(qc, ps_o, "pso", jjs=range(0, 2))
                    )
                    extra = wo_ops(qc, ps_s, "pss", act_cast=False, jjs=range(2, 4))
                    pend_norm.extend(extra)
                    norm_stat["queued"] += len(extra)
            while pend_norm or pend_wo:
                drain(1)
                drain_wo(1)

            ps_o_cm.__exit__(None, None, None)
            ps_s_cm.__exit__(None, None, None)
    _split_multi_waits(nc)
    return nc


_NC_CACHE = {}


def _analyze_mask(mask):
    """Per (kt, qc) tile info from the [S, S] additive mask ([q, k]).

    Returns (tinfo, total_w, blocks): tinfo[kt][qc] = (cls, c0, c1, moff);
    blocks is the list of [128, w] transposed mask blocks to concatenate."""
    masked = mask <= NEG_THRESH  # [q, k] bool
    tinfo = [[None] * NQC for _ in range(NKT)]
    blocks = []
    moff = 0

    def add_block(q0, q1, kt):
        blk = np.asarray(
            mask[q0:q1, kt * 128 : (kt + 1) * 128].T, dtype=np.float32
        )
        blk = np.maximum(blk, -30000.0)
        blocks.append(blk)
        return blk.shape[1]

    for kt in range(NKT):
        for qc in range(NQC):
            sub = masked[qc * 512 : (qc + 1) * 512, kt * 128 : (kt + 1) * 128]
            col_all = sub.all(axis=1)  # q col fully masked
            col_any = sub.any(axis=1)
            if col_all.all():
                tinfo[kt][qc] = ("n", 0, 0, 0)
                continue
            c0 = int(np.argmin(col_all))  # first not-fully-masked col
            assert not col_all[c0:].any(), "non-contiguous masked col range"
            if col_any[c0:].any():
                c1 = 512 - int(np.argmax(col_any[::-1]))  # last any-masked col + 1
            else:
                c1 = c0
            if c1 > c0:
                w = add_block(qc * 512 + c0, qc * 512 + c1, kt)
                tinfo[kt][qc] = ("t", c0, c1, moff)
                moff += w
            else:
                tinfo[kt][qc] = ("t", c0, c0, 0)
    # the first active tile of each q chunk must be full width (it carries
    # the PSUM has_written init for scores and PV)
    for qc in range(NQC):
        acts = [kt for kt in range(NKT) if tinfo[kt][qc][0] != "n"]
        assert acts, "fully-masked q chunk unsupported"
        kt0 = acts[0]
        cls, c0, c1, _ = tinfo[kt0][qc]
        if c0 != 0:
            c1 = max(c1, c0)
            w = add_block(qc * 512, qc * 512 + c1, kt0)
            tinfo[kt0][qc] = ("t", 0, c1, moff)
            moff += w
    return tinfo, moff, blocks


def _prep_inputs(x, freqs_cos, freqs_sin, mask, wq, wk, wv, wo, blocks, total_w):
    bf = ml_dtypes.bfloat16
    x32 = np.asarray(x, dtype=np.float32)  # [B, S, DIM]

    cos = np.asarray(freqs_cos, dtype=np.float32)  # [S, 32]
    sin = np.asarray(freqs_sin, dtype=np.float32)
    d = np.arange(128)
    pair = (d % 64) // 2
    cosd = np.ascontiguousarray(cos[:, pair].T).astype(bf)  # [128, S]
    sgn = np.where(d % 2 == 0, -1.0, 1.0).astype(np.float32)
    sind = np.ascontiguousarray(sin[:, pair].T * sgn[:, None]).astype(bf)

    perm = np.zeros((128, 128), dtype=np.float32)
    idx = np.arange(128)
    perm[idx ^ 1, idx] = 1.0
    perm = perm.astype(bf)
    eye128 = np.eye(128, dtype=np.float32).astype(bf)

    MW = max(total_w, 128)
    mm = np.zeros((128, MW), dtype=np.float32)
    off = 0
    for blk in blocks:
        mm[:, off : off + blk.shape[1]] = blk
        off += blk.shape[1]
    maskmix = np.ascontiguousarray(mm).astype(bf)

    wq = np.asarray(wq, dtype=np.float32) * 0.125  # fold 1/sqrt(HEAD_DIM)
    wk = np.asarray(wk, dtype=np.float32)
    wv = np.asarray(wv, dtype=np.float32)
    wo = np.asarray(wo, dtype=np.float32)

    # host-tile x per batch: [DIM, S] -> [p, nt, kt, tok'] flat
    xts = []
    for b in range(B):
        xT = x32[b].T  # [DIM, S]
        xt = (
            xT.reshape(NKT, 128, NQC, 512)
            .transpose(1, 2, 0, 3)
            .reshape(128, NQC * NKT * 512)
        )
        xts.append(np.ascontiguousarray(xt).astype(bf))

    in_maps = []
    for c in range(N_CORES):
        b, t = divmod(c, TP)
        hs = slice(t * HPC * HEAD_DIM, (t + 1) * HPC * HEAD_DIM)
        kvparts = []
        for j in range(KVPC):
            ks = slice((KVPC * t + j) * HEAD_DIM, (KVPC * t + j + 1) * HEAD_DIM)
            kvparts.append(wk[:, ks])
            kvparts.append(wv[:, ks])
        wkv = np.concatenate(kvparts, axis=1)  # [DIM, 256]
        # host-tile weights to [p, kt, m] (contiguous per-kt chunks)
        wqt = (
            wq[:, hs]
            .reshape(NKT, 128, HPC * HEAD_DIM)
            .transpose(1, 0, 2)
            .reshape(128, NKT * HPC * HEAD_DIM)
        )
        wkvt = (
            wkv.reshape(NKT, 128, KVPC * 2 * HEAD_DIM)
            .transpose(1, 0, 2)
            .reshape(128, NKT * KVPC * 2 * HEAD_DIM)
        )
        in_maps.append(
            {
                "xt_c": xts[b],
                "wq_c": np.ascontiguousarray(wqt).astype(bf),
                "wkv_c": np.ascontiguousarray(wkvt).astype(bf),
                "wo_c": np.ascontiguousarray(wo[hs, :]).astype(bf),
                "maskmix": maskmix,
                "cosd": cosd,
                "sind": sind,
                "perm": perm,
                "eye128": eye128,
            }
        )
    return in_maps


def kernel(x, freqs_cos, freqs_sin, mask, wq, wk, wv, wo, _trace=False):
    tinfo, total_w, blocks = _analyze_mask(np.asarray(mask, dtype=np.float32))
    key = tuple(tuple(r) for r in tinfo)
    if key not in _NC_CACHE:
        _NC_CACHE[key] = build_nc(tinfo, total_w)
    nc = _NC_CACHE[key]
    in_maps = _prep_inputs(
        x, freqs_cos, freqs_sin, mask, wq, wk, wv, wo, blocks, total_w
    )
    res = run_bass_kernel_spmd(
        nc, in_maps, core_ids=list(range(N_CORES)), trace=_trace
    )
    out = np.zeros((B, S, DIM), dtype=np.float32)
    for c in range(N_CORES):
        b = c // TP
        out[b] += np.asarray(res.results[c]["out_c"], dtype=np.float32)
    if _trace:
        kernel._last_exec_time_ns = res.exec_time_ns
        kernel._last_profile_json = res.profile_json
    return out


# revision 7
# speedup vs baseline: 1.1497x; 1.0139x over previous
"""Self-contained Trainium2 Bass kernel for the GQA attention module (v2).

Sharding: 4-way head tensor-parallel x 2-way batch data-parallel.
Core c = 4*b + t owns batch b, q-heads [8t..8t+8), kv-heads {2t, 2t+1};
the host sums the 4 TP partials per batch (the "all-reduce after wo").

v2 design (vs the ~455-539us v1 8-way-TP baseline):
  - Projections and attention are interleaved per 512-token block: project
    block nt, then run the two attention units (one per kv head) for
    q-chunk nt.  The PE-dense projection matmuls fill the ACT-heavy
    attention stretches, so the PE never idles long enough for the HAM
    clock gate to re-throttle it to half rate.
  - Causal column restriction: per (kt, qc) score tile only the q columns
    with any unmasked row are computed (scores, exp, PV all restricted).
  - The additive mask is applied on the PE: an identity-weighted matmul
    accumulates the raw mask block into the score PSUM before exp, so exp
    writes straight to the P^T ring and the DVE mask multiplies are gone.
  - The PV accumulator (pso) is copied to SBUF right after the last PV
    matmul, releasing its 4 PSUM banks early so the next unit's PV is
    never blocked (pso pool has bufs=1; all other PSUM transients share
    one 2-deep [128,1024] ring).
  - wo runs per q-chunk (needs both kv-head units' at); its matmuls and
    the normalize chain are deferred closures drained at a paced rate
    during the following projection/attention slots, as in v1.
"""

import sys
import types

sys.path.insert(0, "/opt/trn_rl_repo")

import numpy as np
import ml_dtypes


def _install_axon_hook_shim():
    import antenv

    if "antenv.axon_hooks" in sys.modules:
        return
    m = types.ModuleType("antenv.axon_hooks")
    m._hook = None

    def set_axon_ntff_profile_hook(h):
        m._hook = h

    def get_axon_ntff_profile_hook():
        return m._hook

    m.set_axon_ntff_profile_hook = set_axon_ntff_profile_hook
    m.get_axon_ntff_profile_hook = get_axon_ntff_profile_hook
    sys.modules["antenv.axon_hooks"] = m
    antenv.axon_hooks = m
    try:
        from trn_agent_boot.trn_boot import _ntff_profile_via_ctypes

        hook = _ntff_profile_via_ctypes("/opt/axon/libaxon_pjrt.so")
        if hook is not None:
            m.set_axon_ntff_profile_hook(hook)
    except Exception:
        pass


_install_axon_hook_shim()

import concourse.bass as bass
import concourse.mybir as mybir
import concourse.tile as tile
from concourse.bass_utils import run_bass_kernel_spmd

BF16 = mybir.dt.bfloat16
F16 = mybir.dt.float16
F32 = mybir.dt.float32

B, S, DIM = 2, 2048, 2048
N_HEADS, N_KV_HEADS, HEAD_DIM = 32, 8, 64
N_CORES = 8
TP = 4  # head-parallel groups
HPC = N_HEADS // TP  # 8 q heads per core
KVPC = N_KV_HEADS // TP  # 2 kv heads per core
NPAIR = HPC // 2  # 4 head pairs per core
TOK = S  # tokens per core (its batch)
NKT = S // 128  # 16 k tiles
NQC = S // 512  # 4 q chunks
NEG_THRESH = -1e4


def _patched_drain_and_barrier(self, tick_clock, wait_clock):
    # walrus (CoreV3) only accepts one sync-wait on the tile exit drain;
    # split the accumulated waits across single-wait nops.
    nc = self.nc
    drain_inst = nc.sync.drain()
    wait_clock.add_sem_waits(
        drain_inst.ins, tile.ScopedClock({None: tick_clock.global_clock})
    )
    si = drain_inst.ins.sync_info
    sw = list(si.on_wait) if si and si.on_wait else []
    if len(sw) > 1:
        si.on_wait = [sw[0]]
        for w in sw[1:]:
            n2 = nc.sync.nop(nofuse=True)
            if n2.ins.sync_info is None:
                n2.ins.sync_info = mybir.SyncInfo(on_wait=[w], on_update=[])
            else:
                n2.ins.sync_info.on_wait = [w]
    nc.all_engine_barrier()
    assert self.sems is not None
    popped = nc._tile_sem_poison_stack.pop()
    assert popped is self._sem_poison
    nc.clear_and_free_semaphores(list(self.sems.allocated().values()))
    nc.all_engine_barrier()


tile.TileContext._drain_and_barrier = _patched_drain_and_barrier


def _split_multi_waits(nc):
    """walrus (this build) accepts at most one sync-wait per instruction;
    move extra waits onto same-engine nops inserted just before."""
    n_split = 0
    for f in nc.m.functions:
        for blk in f.blocks:
            new_insts = []
            for inst in blk.instructions:
                si = getattr(inst, "sync_info", None)
                if si is not None and si.on_wait and len(si.on_wait) > 1:
                    extra = list(si.on_wait[:-1])
                    si.on_wait = [si.on_wait[-1]]
                    for w in extra:
                        nop = mybir.InstNoOp(
                            name=nc.get_next_instruction_name(), ins=[], outs=[]
                        )
                        nop.engine = inst.engine
                        nop.sync_info = mybir.SyncInfo(on_wait=[w], on_update=[])
                        new_insts.append(nop)
                        n_split += 1
                new_insts.append(inst)
            blk.instructions[:] = new_insts
    return n_split


def build_nc(tinfo, mask_total_w):
    """tinfo[kt][qc] = ('n'|'t', c0, c1, moff): fully-masked / take, with
    computed q-col range [c0, 512), mixed mask cols [c0, c1) at maskmix
    offset moff (c1 == c0 means no mask needed)."""
    nc = bass.Bass("TRN2", target_bir_lowering=False, debug=False, num_devices=N_CORES)

    MW = max(mask_total_w, 128)
    # x is host-tiled to [p, nt, kt, tok'] so each 512-token block is one
    # contiguous-per-partition 16 KiB-line DMA
    xt_d = nc.dram_tensor("xt_c", [128, NQC * NKT * 512], BF16, kind="ExternalInput")
    # wq/wkv are host-tiled to [p, kt, m] so per-kt chunks are contiguous
    wq_d = nc.dram_tensor(
        "wq_c", [128, NKT * HPC * HEAD_DIM], BF16, kind="ExternalInput"
    )
    wkv_d = nc.dram_tensor(
        "wkv_c", [128, NKT * KVPC * 2 * HEAD_DIM], BF16, kind="ExternalInput"
    )
    wo_d = nc.dram_tensor("wo_c", [HPC * HEAD_DIM, DIM], BF16, kind="ExternalInput")
    maskmix_d = nc.dram_tensor("maskmix", [128, MW], BF16, kind="ExternalInput")
    cosd_d = nc.dram_tensor("cosd", [128, TOK], BF16, kind="ExternalInput")
    sind_d = nc.dram_tensor("sind", [128, TOK], BF16, kind="ExternalInput")
    perm_d = nc.dram_tensor("perm", [128, 128], BF16, kind="ExternalInput")
    eye128_d = nc.dram_tensor("eye128", [128, 128], BF16, kind="ExternalInput")
    out_d = nc.dram_tensor("out_c", [TOK, DIM], BF16, kind="ExternalOutput")

    with tile.TileContext(nc) as tc:
        with (
            tc.tile_pool(name="persist", bufs=1) as persist,
            tc.tile_pool(name="stream", bufs=2) as stream,
            tc.tile_pool(name="small", bufs=2) as small,
            tc.tile_pool(name="otp", bufs=2) as otp,
            tc.tile_pool(name="bcp", bufs=2) as bcp,
        ):
            # ---- persistent tensors ----
            wq_sb = persist.tile([128, NKT, HPC * HEAD_DIM], BF16, tag="wq")
            wkv_sb = persist.tile([128, NKT, KVPC * 2 * HEAD_DIM], BF16, tag="wkv")
            wo_sb = persist.tile([128, NPAIR, DIM], BF16, tag="wo")
            perm_sb = persist.tile([128, 128], BF16, tag="perm")
            eye_sb = persist.tile([128, 128], BF16, tag="eye")
            cos_sb = persist.tile([128, TOK], BF16, tag="cos")
            sin_sb = persist.tile([128, TOK], BF16, tag="sin")
            mask_sb = persist.tile([128, MW], BF16, tag="mask")
            q_sb = persist.tile([128, NPAIR, TOK], BF16, tag="q")  # Q^T
            kT_sb = persist.tile([128, KVPC, TOK], BF16, tag="kT")  # K^T dup halves
            v_sb = persist.tile([128, KVPC * NKT, 68], BF16, tag="v")  # [V|1|pad]
            at_sb = persist.tile([128, NPAIR, TOK], BF16, tag="at")  # normalized A^T
            # raw A^T ring (row 64 carries the softmax denominators)
            aout_sb = persist.tile([65, 2, 4, 512], BF16, tag="aout")
            rec32_sb = persist.tile([65, 2048], F16, tag="rec32")  # ln d (row 64)
            recb_sb = persist.tile([65, 2048], BF16, tag="recb")  # 1/d (row 64)
            ones_sb = persist.tile([128, 64], BF16, tag="ones")
            NRING = 5
            pring = persist.tile([128, NRING, 2048], BF16, tag="pring")

            # DMA emission in need-order: the first Q matmul needs xblk0[kt]
            # and the mt=0 slice of wq[kt], so those chunks go first.
            xblk0 = stream.tile([128, NKT, 512], BF16, tag="xblk")
            for g in range(4):
                nc.gpsimd.dma_start(
                    xblk0[:, 4 * g : 4 * g + 4, :].rearrange("p t n -> p (t n)"),
                    xt_d[:, g * 2048 : (g + 1) * 2048],
                )
                # kv weights first: the kv tiles are projected first
                nc.sync.dma_start(
                    wkv_sb[:, 4 * g : 4 * g + 4, :].rearrange("p t n -> p (t n)"),
                    wkv_d[:, g * 1024 : (g + 1) * 1024],
                )
                nc.sync.dma_start(
                    wq_sb[:, 4 * g : 4 * g + 4, :].rearrange("p t n -> p (t n)"),
                    wq_d[:, g * 2048 : (g + 1) * 2048],
                )
            nc.sync.dma_start(perm_sb[:], perm_d[:])
            nc.sync.dma_start(eye_sb[:], eye128_d[:])
            nc.sync.dma_start(cos_sb[:], cosd_d[:])
            nc.sync.dma_start(sin_sb[:], sind_d[:])
            nc.sync.dma_start(mask_sb[:], maskmix_d[:])
            nc.sync.dma_start(wo_sb[:], wo_d.rearrange("(t p) m -> p t m", p=128))
            nc.gpsimd.memset(v_sb[:, :, 64:65], 1.0)
            nc.gpsimd.memset(ones_sb[:], 1.0)

            # PSUM: one 2-deep [128,1024] ring for every transient (score
            # pairs, projection accumulators, RoPE swaps, V-transpose, wo
            # chunks, 1/d broadcast) + a single 4-bank PV accumulator.
            ps_s_cm = tc.tile_pool(name="ps_s", bufs=2, space="PSUM")
            ps_s = ps_s_cm.__enter__()
            ps_o_cm = tc.tile_pool(name="ps_o", bufs=1, space="PSUM")
            ps_o = ps_o_cm.__enter__()

            # Two deferred-closure queues: light normalize work (pss-tag /
            # DVE) drains anywhere; wo chunks allocate the 4-bank pso slot
            # and may ONLY drain where pso is free (projection tile windows
            # and unit tails) — draining one mid-unit would deadlock the PE
            # queue behind the unit's own pso.
            pend_norm = []
            pend_wo = []  # entries: (need, closure) — need = norm ops that
            # must have drained first (the at_sb writes wo reads)
            norm_stat = {"queued": 0, "drained": 0}

            def drain(k):
                for _ in range(min(k, len(pend_norm))):
                    pend_norm.pop(0)()
                    norm_stat["drained"] += 1

            def drain_wo(k, force=False):
                for _ in range(min(k, len(pend_wo))):
                    need, rel, op = pend_wo[0]
                    if norm_stat["drained"] < need:
                        if not force:
                            return
                        while norm_stat["drained"] < need and pend_norm:
                            pend_norm.pop(0)()
                            norm_stat["drained"] += 1
                        if norm_stat["drained"] < need:
                            return
                    pend_wo.pop(0)
                    op()

            def close_wo():
                # an op1 without its op2 holds the pso slot; close the pair
                # before emitting an attention unit (else its PV deadlocks
                # behind the unreleased slot in the PE queue)
                if pend_wo and pend_wo[0][1] == 1:
                    drain_wo(1, force=True)

            # ---------------- projection of one 512-token block ----------
            def project_block(nt, xblk=None):
                cs = slice(nt * 512, (nt + 1) * 512)
                if xblk is None:
                    xblk = stream.tile([128, NKT, 512], BF16, tag="xblk")
                    base = nt * NKT * 512
                    for g in range(4):
                        nc.sync.dma_start(
                            xblk[:, 4 * g : 4 * g + 4, :].rearrange(
                                "p t n -> p (t n)"
                            ),
                            xt_d[:, base + g * 2048 : base + (g + 1) * 2048],
                        )
                cosb = cos_sb[:, cs]
                sinb = sin_sb[:, cs]

                # Projection tiles are software-pipelined: tile t's RoPE tail
                # (perm matmul + muls) is emitted after tile t+1's
                # accumulation matmuls, so the PE never queues behind the
                # PSUM->SBUF copy.  The pair-swap matmul writes into the
                # accumulator tile's unused second bank (no extra slot).
                def rope_q(psq, mt):
                    q_tmp = small.tile([128, 512], BF16, tag="q_tmp")
                    nc.scalar.copy(q_tmp[:], psq[:, 0:512])
                    nc.tensor.matmul(psq[:, 512:1024], perm_sb[:], q_tmp[:])
                    v1 = small.tile([128, 512], BF16, tag="v1")
                    nc.vector.tensor_mul(v1[:], q_tmp[:], cosb)
                    v2 = small.tile([128, 512], BF16, tag="v2")
                    nc.vector.tensor_mul(v2[:], psq[:, 512:1024], sinb)
                    nc.vector.tensor_add(q_sb[:, mt, cs], v1[:], v2[:])

                def rope_kv(pskv, j):
                    kv_tmp = small.tile([128, 512], BF16, tag="kv_tmp")
                    nc.scalar.copy(kv_tmp[:], pskv[:, 0:512])
                    # K RoPE on rows 0:64
                    nc.tensor.matmul(
                        pskv[0:64, 512:1024], perm_sb[0:64, 0:64], kv_tmp[0:64, :]
                    )
                    kv1 = small.tile([64, 512], BF16, tag="kv1")
                    nc.vector.tensor_mul(kv1[:], kv_tmp[0:64, :], cosb[0:64, :])
                    kv2 = small.tile([64, 512], BF16, tag="kv2")
                    nc.vector.tensor_mul(kv2[:], pskv[0:64, 512:1024], sinb[0:64, :])
                    nc.vector.tensor_add(kT_sb[0:64, j, cs], kv1[:], kv2[:])
                    # duplicate K^T into partitions 64..127 (so the row-packed
                    # score matmul pairs get distinct PE row groups)
                    nc.gpsimd.dma_start(kT_sb[64:128, j, cs], kT_sb[0:64, j, cs])

                    # V transpose: rows 64:128 of kv_tmp -> natural V [k, 64]
                    pst = ps_s.tile([128, 4, 64], BF16, tag="pss", name="pst")
                    for jq in range(4):
                        nc.tensor.transpose(
                            pst[:, jq, :],
                            kv_tmp[64:128, jq * 128 : (jq + 1) * 128],
                            eye_sb[64:128, 64:128],
                            tile_position=(64, 0),
                        )
                    rc0 = j * NKT + nt * 4
                    nc.scalar.copy(v_sb[:, rc0 : rc0 + 4, 0:64], pst[:])

                st = {"tail": None}

                def do_tiles(tiles):
                    for kind, idx in tiles:
                        ps = ps_s.tile([128, 1024], F32, tag="pss", name="psp")
                        wsb = wq_sb if kind == "q" else wkv_sb
                        for kt in range(NKT):
                            nc.tensor.matmul(
                                ps[:, 0:512],
                                wsb[:, kt, idx * 128 : (idx + 1) * 128],
                                xblk[:, kt, :],
                                start=(kt == 0),
                                stop=(kt == NKT - 1),
                            )
                        if st["tail"]:
                            st["tail"]()
                        if kind == "q":
                            st["tail"] = lambda ps=ps, idx=idx: rope_q(ps, idx)
                        else:
                            st["tail"] = lambda ps=ps, idx=idx: rope_kv(ps, idx)
                        drain(2)
                        drain_wo(2)

                def flush():
                    if st["tail"]:
                        st["tail"]()
                        st["tail"] = None

                return do_tiles, flush

            # ---------------- one attention unit: (qc, kv head j) --------
            def attn_unit(qc, j, uidx, wo_slots=0):
                acts = [kt for kt in range(NKT) if tinfo[kt][qc][0] != "n"]
                assert acts, "fully-masked q chunk unsupported"
                ring = aout_sb[:, uidx % 2]  # [64, 4, 512]
                u = {"qc": qc, "j": j, "ring": ring, "pso": None}
                # wo chunks on the pso slot may drain here only when this
                # unit follows a projection block (the previous unit's pso
                # has long been released, so the PE queue won't block);
                # complete pairs only, and never leave a half-open chunk
                close_wo()
                drain_wo(wo_slots, force=True)
                close_wo()

                def emit_pv(i):
                    kt = acts[i]
                    c0 = tinfo[kt][qc][1]
                    rc = j * NKT + kt
                    for h in range(4):
                        nc.tensor.matmul(
                            u["pso"][:, h, c0:512],
                            v_sb[:, rc, 0:65],
                            pring[:, i % NRING, h * 512 + c0 : (h + 1) * 512],
                            start=(i == 0),
                            stop=(i == len(acts) - 1),
                        )

                for i, kt in enumerate(acts):
                    cls, c0, c1, moff = tinfo[kt][qc]
                    if i == 0:
                        assert c0 == 0, "first active kt must be full-width"
                    ks = slice(kt * 128, (kt + 1) * 128)
                    mw = c1 - c0
                    # scores for 4 heads: 2 row-group-packed matmul pairs
                    for pair in range(2):
                        mt = 2 * j + pair
                        qs = slice(qc * 512 + c0, (qc + 1) * 512)
                        pss = ps_s.tile([128, 1024], F32, tag="pss", name="pss")
                        nc.tensor.matmul(
                            pss[:, c0:512],
                            kT_sb[0:64, j, ks],
                            q_sb[0:64, mt, qs],
                            tile_position=(0, 0),
                            start=True,
                            stop=(mw == 0),
                        )
                        nc.tensor.matmul(
                            pss[:, 512 + c0 : 1024],
                            kT_sb[64:128, j, ks],
                            q_sb[64:128, mt, qs],
                            tile_position=(64, 0),
                            start=True,
                            stop=(mw == 0),
                        )
                        if mw:
                            # accumulate the raw additive mask block into the
                            # mixed columns on the PE (identity stationary;
                            # two plain-AP matmuls — a broadcast moving
                            # operand here loses the RAW dep on the mask DMA)
                            nc.tensor.matmul(
                                pss[:, c0:c1],
                                eye_sb[:],
                                mask_sb[:, moff : moff + mw],
                                start=False,
                                stop=True,
                            )
                            nc.tensor.matmul(
                                pss[:, 512 + c0 : 512 + c1],
                                eye_sb[:],
                                mask_sb[:, moff : moff + mw],
                                start=False,
                                stop=True,
                            )
                        nc.scalar.activation(
                            pring[:, i % NRING, pair * 1024 : (pair + 1) * 1024]
                            .rearrange("p (t n) -> p t n", t=2)[:, :, c0:512],
                            pss[:].rearrange("p (t n) -> p t n", t=2)[:, :, c0:512],
                            mybir.ActivationFunctionType.Exp,
                        )
                    # drain light closures, paced across the sweep
                    k = (
                        -(-len(pend_norm) // max(1, len(acts) - i))
                        if pend_norm
                        else 0
                    )
                    drain(min(k, 2))
                    if i == 1:
                        u["pso"] = ps_o.tile([65, 4, 512], F32, tag="pso", name="pso")
                    if i >= 2:
                        emit_pv(i - 2)
                if len(acts) == 1:
                    u["pso"] = ps_o.tile([65, 4, 512], F32, tag="pso", name="pso")
                drain(len(pend_norm))
                for i in range(max(0, len(acts) - 2), len(acts)):
                    emit_pv(i)

                # release pso quickly: copy raw A^T plus the denominator row
                # (65 partitions) to SBUF, split across DVE and ACT.  The
                # ln/exp reciprocal chain runs later as deferred closures so
                # it never delays the next unit's exps in the ACT FIFO.
                pso = u["pso"]
                nc.vector.tensor_copy(ring[:, :, :], pso[:, :, :])
                return u

            def norm_ops(u):
                """Deferred normalize closures for unit u (reads aout ring)."""
                qc, j, ring = u["qc"], u["j"], u["ring"]
                qcs = slice(qc * 512, (qc + 1) * 512)
                ops = []

                def op_ln():
                    nc.scalar.activation(
                        rec32_sb[64:65, :],
                        ring[64:65, :, :].rearrange("p a n -> p (a n)"),
                        mybir.ActivationFunctionType.Ln,
                    )

                def op_recip():
                    nc.scalar.activation(
                        recb_sb[64:65, :],
                        rec32_sb[64:65, :],
                        mybir.ActivationFunctionType.Exp,
                        scale=-1.0,
                    )

                ops.append(op_ln)
                ops.append(op_recip)
                psbs = [None, None]
                for p in range(2):  # head pair within this kv head
                    ch = 2 * j + p

                    def op_psb(p=p):
                        # partition-broadcast 1/d to 64 rows via a K=1 matmul
                        psb = ps_s.tile([128, 1024], F32, tag="pss", name="psb")
                        psbs[p] = psb
                        for hh in range(2):
                            h = 2 * p + hh
                            nc.tensor.matmul(
                                psb[0:64, hh * 512 : (hh + 1) * 512],
                                ones_sb[64:65, :],
                                recb_sb[64:65, h * 512 : (h + 1) * 512],
                            )

                    def op_mul(p=p, ch=ch):
                        bc = bcp.tile([64, 1024], BF16, tag="bc")
                        nc.vector.tensor_copy(bc[:], psbs[p][0:64, :])
                        nc.vector.tensor_mul(
                            at_sb[0:64, ch, qcs], ring[0:64, 2 * p, :], bc[:, 0:512]
                        )
                        att = small.tile([64, 512], BF16, tag="att")
                        nc.vector.tensor_mul(
                            att[:], ring[0:64, 2 * p + 1, :], bc[:, 512:1024]
                        )
                        nc.gpsimd.dma_start(at_sb[64:128, ch, qcs], att[:])

                    ops.append(op_psb)
                    ops.append(op_mul)
                return ops

            def wo_ops(qc, pool, tag, act_cast=True, jjs=range(4)):
                """wo chunk closures for q-chunk qc, split in two gated
                halves: ch 0-1 matmuls need only unit (qc,0)'s at, ch 2-3
                plus the cast need unit (qc,1)'s.  Returns (rel, closure)
                with rel 0/1 = which unit's normalize ops must be drained."""
                base = qc * 512
                ops = []
                for jj in jjs:
                    rs = slice(base + jj * 128, base + (jj + 1) * 128)
                    for half in range(2):
                        cell = [None]

                        def op1(rs=rs, half=half, cell=cell):
                            psd = pool.tile([128, 1024], F32, tag=tag, name="psd")
                            cell[0] = psd
                            for sub in range(2):
                                cs2 = slice(
                                    (half * 2 + sub) * 512,
                                    (half * 2 + sub + 1) * 512,
                                )
                                for ch in range(2):
                                    nc.tensor.matmul(
                                        psd[:, sub * 512 : (sub + 1) * 512],
                                        at_sb[:, ch, rs],
                                        wo_sb[:, ch, cs2],
                                        start=(ch == 0),
                                        stop=False,
                                    )

                        def op2(rs=rs, half=half, cell=cell):
                            psd = cell[0]
                            for sub in range(2):
                                cs2 = slice(
                                    (half * 2 + sub) * 512,
                                    (half * 2 + sub + 1) * 512,
                                )
                                for ch in range(2, NPAIR):
                                    nc.tensor.matmul(
                                        psd[:, sub * 512 : (sub + 1) * 512],
                                        at_sb[:, ch, rs],
                                        wo_sb[:, ch, cs2],
                                        start=False,
                                        stop=(ch == NPAIR - 1),
                                    )
                            ot = otp.tile([128, 1024], BF16, tag="ot")
                            # alternate the PSUM drain between DVE and ACT so
                            # two casts can be in flight (DVE-only when the
                            # chunks drain mid-attention, where ACT is busy)
                            if half == 1 and act_cast:
                                nc.scalar.copy(ot[:], psd[:])
                            else:
                                nc.vector.tensor_copy(ot[:], psd[:])
                            nc.sync.dma_start(
                                out_d[rs, half * 1024 : (half + 1) * 1024], ot[:]
                            )

                        ops.append((0, op1))
                        ops.append((1, op2))
                return ops

            # ---------------- main interleaved schedule ------------------
            # Projections run one block ahead of attention, split in halves
            # around the first attention unit of each chunk so every unit
            # start follows PE-dense projection work (keeps HAM warm and
            # lets the previous unit's exp backlog drain).  Early q-chunks'
            # wo uses the pso slot (drained in projection windows and at
            # post-projection unit starts); late chunks fall back to the
            # shared pss ring and drain mid-unit.
            HALF1 = [("q", 0), ("q", 1), ("q", 2)]
            HALF2 = [("q", 3), ("kv", 0), ("kv", 1)]
            uidx = 0
            do0, fl0 = project_block(0, xblk=xblk0)
            # kv tiles first for block 0: their weight tensor is smaller, so
            # the very first accumulation starts sooner after the DMAs
            do0([("kv", 0), ("kv", 1)] + HALF1[:3] + [("q", 3)])
            fl0()
            for qc in range(NQC):
                do = fl = None
                if qc + 1 < NQC:
                    do, fl = project_block(qc + 1)
                    do(HALF1)
                u = attn_unit(qc, 0, uidx, wo_slots=2)
                uidx += 1
                pend_norm.extend(norm_ops(u))
                norm_stat["queued"] += 6
                need0 = norm_stat["queued"]
                if do:
                    do(HALF2)
                    fl()
                u = attn_unit(qc, 1, uidx, wo_slots=(2 if do else 0))
                uidx += 1
                pend_norm.extend(norm_ops(u))
                norm_stat["queued"] += 6
                need1 = norm_stat["queued"]
                if qc < 2:
                    pend_wo.extend(
                        (need0 if rel == 0 else need1, rel, op)
                        for rel, op in wo_ops(qc, ps_o, "pso")
                    )
                else:
                    # half on the pso slot (drains at the remaining unit
                    # starts / final drain), half on the shared ring so two
                    # psd streams can be in flight at the end
                    pend_wo.extend(
                        (need0 if rel == 0 else need1, rel, op)
                        for rel, op in wo_ops(qc, ps_o, "pso", jjs=range(0, 2))
                    )
                    extra = [
                        op
                        for _, op in wo_ops(
                            qc, ps_s, "pss", act_cast=(qc == 3), jjs=range(2, 4)
                        )
                    ]
                    pend_norm.extend(extra)
                    norm_stat["queued"] += len(extra)
            while pend_norm or pend_wo:
                drain(1)
                drain_wo(1, force=True)

            ps_o_cm.__exit__(None, None, None)
            ps_s_cm.__exit__(None, None, None)
    _split_multi_waits(nc)
    return nc


_NC_CACHE = {}


def _analyze_mask(mask):
    """Per (kt, qc) tile info from the [S, S] additive mask ([q, k]).

    Returns (tinfo, total_w, blocks): tinfo[kt][qc] = (cls, c0, c1, moff);
    blocks is the list of [128, w] transposed mask blocks to concatenate."""
    masked = mask <= NEG_THRESH  # [q, k] bool
    tinfo = [[None] * NQC for _ in range(NKT)]
    blocks = []
    moff = 0

    def add_block(q0, q1, kt):
        blk = np.asarray(
            mask[q0:q1, kt * 128 : (kt + 1) * 128].T, dtype=np.float32
        )
        blk = np.maximum(blk, -30000.0)
        blocks.append(blk)
        return blk.shape[1]

    for kt in range(NKT):
        for qc in range(NQC):
            sub = masked[qc * 512 : (qc + 1) * 512, kt * 128 : (kt + 1) * 128]
            col_all = sub.all(axis=1)  # q col fully masked
            col_any = sub.any(axis=1)
            if col_all.all():
                tinfo[kt][qc] = ("n", 0, 0, 0)
                continue
            c0 = int(np.argmin(col_all))  # first not-fully-masked col
            assert not col_all[c0:].any(), "non-contiguous masked col range"
            if col_any[c0:].any():
                c1 = 512 - int(np.argmax(col_any[::-1]))  # last any-masked col + 1
            else:
                c1 = c0
            if c1 > c0:
                w = add_block(qc * 512 + c0, qc * 512 + c1, kt)
                tinfo[kt][qc] = ("t", c0, c1, moff)
                moff += w
            else:
                tinfo[kt][qc] = ("t", c0, c0, 0)
    # the first active tile of each q chunk must be full width (it carries
    # the PSUM has_written init for scores and PV)
    for qc in range(NQC):
        acts = [kt for kt in range(NKT) if tinfo[kt][qc][0] != "n"]
        assert acts, "fully-masked q chunk unsupported"
        kt0 = acts[0]
        cls, c0, c1, _ = tinfo[kt0][qc]
        if c0 != 0:
            c1 = max(c1, c0)
            w = add_block(qc * 512, qc * 512 + c1, kt0)
            tinfo[kt0][qc] = ("t", 0, c1, moff)
            moff += w
    return tinfo, moff, blocks


def _prep_inputs(x, freqs_cos, freqs_sin, mask, wq, wk, wv, wo, blocks, total_w):
    bf = ml_dtypes.bfloat16
    x32 = np.asarray(x, dtype=np.float32)  # [B, S, DIM]

    cos = np.asarray(freqs_cos, dtype=np.float32)  # [S, 32]
    sin = np.asarray(freqs_sin, dtype=np.float32)
    d = np.arange(128)
    pair = (d % 64) // 2
    cosd = np.ascontiguousarray(cos[:, pair].T).astype(bf)  # [128, S]
    sgn = np.where(d % 2 == 0, -1.0, 1.0).astype(np.float32)
    sind = np.ascontiguousarray(sin[:, pair].T * sgn[:, None]).astype(bf)

    perm = np.zeros((128, 128), dtype=np.float32)
    idx = np.arange(128)
    perm[idx ^ 1, idx] = 1.0
    perm = perm.astype(bf)
    eye128 = np.eye(128, dtype=np.float32).astype(bf)

    MW = max(total_w, 128)
    mm = np.zeros((128, MW), dtype=np.float32)
    off = 0
    for blk in blocks:
        mm[:, off : off + blk.shape[1]] = blk
        off += blk.shape[1]
    maskmix = np.ascontiguousarray(mm).astype(bf)

    wq = np.asarray(wq, dtype=np.float32) * 0.125  # fold 1/sqrt(HEAD_DIM)
    wk = np.asarray(wk, dtype=np.float32)
    wv = np.asarray(wv, dtype=np.float32)
    wo = np.asarray(wo, dtype=np.float32)

    # host-tile x per batch: [DIM, S] -> [p, nt, kt, tok'] flat
    xts = []
    for b in range(B):
        xT = x32[b].T  # [DIM, S]
        xt = (
            xT.reshape(NKT, 128, NQC, 512)
            .transpose(1, 2, 0, 3)
            .reshape(128, NQC * NKT * 512)
        )
        xts.append(np.ascontiguousarray(xt).astype(bf))

    in_maps = []
    for c in range(N_CORES):
        b, t = divmod(c, TP)
        hs = slice(t * HPC * HEAD_DIM, (t + 1) * HPC * HEAD_DIM)
        kvparts = []
        for j in range(KVPC):
            ks = slice((KVPC * t + j) * HEAD_DIM, (KVPC * t + j + 1) * HEAD_DIM)
            kvparts.append(wk[:, ks])
            kvparts.append(wv[:, ks])
        wkv = np.concatenate(kvparts, axis=1)  # [DIM, 256]
        # host-tile weights to [p, kt, m] (contiguous per-kt chunks)
        wqt = (
            wq[:, hs]
            .reshape(NKT, 128, HPC * HEAD_DIM)
            .transpose(1, 0, 2)
            .reshape(128, NKT * HPC * HEAD_DIM)
        )
        wkvt = (
            wkv.reshape(NKT, 128, KVPC * 2 * HEAD_DIM)
            .transpose(1, 0, 2)
            .reshape(128, NKT * KVPC * 2 * HEAD_DIM)
        )
        in_maps.append(
            {
                "xt_c": xts[b],
                "wq_c": np.ascontiguousarray(wqt).astype(bf),
                "wkv_c": np.ascontiguousarray(wkvt).astype(bf),
                "wo_c": np.ascontiguousarray(wo[hs, :]).astype(bf),
                "maskmix": maskmix,
                "cosd": cosd,
                "sind": sind,
                "perm": perm,
                "eye128": eye128,
            }
        )
    return in_maps


def kernel(x, freqs_cos, freqs_sin, mask, wq, wk, wv, wo, _trace=False):
    tinfo, total_w, blocks = _analyze_mask(np.asarray(mask, dtype=np.float32))
    key = tuple(tuple(r) for r in tinfo)
    if key not in _NC_CACHE:
        _NC_CACHE[key] = build_nc(tinfo, total_w)
    nc = _NC_CACHE[key]
    in_maps = _prep_inputs(
        x, freqs_cos, freqs_sin, mask, wq, wk, wv, wo, blocks, total_w
    )
    res = run_bass_kernel_spmd(
        nc, in_maps, core_ids=list(range(N_CORES)), trace=_trace
    )
    out = np.zeros((B, S, DIM), dtype=np.float32)
    for c in range(N_CORES):
        b = c // TP
        out[b] += np.asarray(res.results[c]["out_c"], dtype=np.float32)
    if _trace:
        kernel._last_exec_time_ns = res.exec_time_ns
        kernel._last_profile_json = res.profile_json
    return out
